# revision 32
# baseline (speedup 1.0000x reference)
"""Trainium2 Bass kernel for nn_MultiHeadAttention_3796751090171 (sparse_attention).

Head-parallel SPMD across 8 NeuronCores: core c computes head c's attention,
then the cores AllGather the (normalized) per-head context vectors and each
core computes a 64-column slice of the output projection — so no cross-core
reduction is ever needed (output = pure concatenation).

Math per head h (core c = h):
  Q = q_feat @ Wq[:, h*64:(h+1)*64] + bq_h          [N, 64]   (pre-scaled 1/8)
  K = k_feat @ Wk_h + bk_h                           [N, 64]
  V = v_feat @ Wv_h + bv_h                           [N, 64]
  S = Q @ K.T + pos_enc[h]   (block-diagonal only)   sparse [N, N]
  P = exp(S); Z = row sums (via ones-column in the V matmul)
  hT = (V|1).T @ expT ; hT /= Z
  -- AllGather hT over heads -> hTf [512, N] --
  outT_c = Wo[:, c*64:(c+1)*64].T @ hTf + bo_c       [64, N]
Host: out[:, c*64:(c+1)*64] = outT_c.T

Sparsity: q_batch/k_batch are SORTED, so the cross-batch mask is block-diagonal
over (q-range x k-range) batch blocks; we only compute those blocks and never
need elementwise masking (k-chunks are batch-aligned).

No max-subtraction in softmax: scores are O(10) so exp is safe in fp32; fully
masked blocks are simply never computed (prob contribution exactly 0, matching
the reference's exp(-1e9 - max) -> 0).
"""

import functools
import math

import numpy as np
import ml_dtypes

import concourse.bass as bass
import concourse.tile as tile
from concourse import bacc, mybir
from concourse.bass_utils import run_bass_kernel_spmd
from concourse.masks import make_identity

N = 3072
QD = 512
OD = 512
H = 8
D = 64
B = 8
NCORES = 8
SCALE = math.sqrt(D)

F32 = mybir.dt.float32
BF16 = mybir.dt.bfloat16
BF16_NP = ml_dtypes.bfloat16

# test.py can flip these to get a profile
TRACE = False
LAST_RESULTS = None


def _plan(q_batch, k_batch):
    """Batch block boundaries from the sorted batch-id vectors."""
    qb = np.asarray(q_batch).astype(np.int64)
    kb = np.asarray(k_batch).astype(np.int64)
    qbound = np.searchsorted(qb, np.arange(B + 1))
    kbound = np.searchsorted(kb, np.arange(B + 1))
    batches = []
    degenerate = False
    for b in range(B):
        q0, q1 = int(qbound[b]), int(qbound[b + 1])
        k0, k1 = int(kbound[b]), int(kbound[b + 1])
        if q1 > q0 and k1 > k0:
            batches.append((q0, q1, k0, k1))
        elif q1 > q0 and k1 == k0:
            # rows with no visible keys: reference gives uniform attention over
            # ALL keys; handled by numpy fallback (never happens in practice)
            degenerate = True
    return tuple(batches), degenerate


def _chunks(lo, hi, step):
    return [(o, min(step, hi - o)) for o in range(lo, hi, step)]


@functools.lru_cache(maxsize=8)
def _build(batches, has_bq, has_bk, has_bv, has_bo):
    nc = bacc.Bacc("TRN2", target_bir_lowering=False, debug=False,
                   num_devices=NCORES)

    # ---- DRAM parameters (per-core values supplied via in_maps) ----
    qfT_d = nc.dram_tensor("qfT", [QD, N], BF16, kind="ExternalInput")
    kfT_d = nc.dram_tensor("kfT", [QD, N], BF16, kind="ExternalInput")
    vfT_d = nc.dram_tensor("vfT", [QD, N], BF16, kind="ExternalInput")
    posT_d = nc.dram_tensor("posT", [N, N], BF16, kind="ExternalInput")
    wq_d = nc.dram_tensor("wq", [QD, D], BF16, kind="ExternalInput")
    wk_d = nc.dram_tensor("wk", [QD, D], BF16, kind="ExternalInput")
    wv_d = nc.dram_tensor("wv", [QD, D], BF16, kind="ExternalInput")
    woc_d = nc.dram_tensor("woc", [OD, D], BF16, kind="ExternalInput")
    bq_d = nc.dram_tensor("bq", [1, D], BF16, kind="ExternalInput") if has_bq else None
    bk_d = nc.dram_tensor("bk", [1, D], BF16, kind="ExternalInput") if has_bk else None
    bv_d = nc.dram_tensor("bv", [1, D], BF16, kind="ExternalInput") if has_bv else None
    boc_d = nc.dram_tensor("boc", [D, 1], F32, kind="ExternalInput") if has_bo else None
    out_d = nc.dram_tensor("out", [D, N], F32, kind="ExternalOutput")

    # global k-chunk list, aligned to batch boundaries (never crosses one)
    kchunk_list = []   # (koff, klen)
    batch_kchunks = []  # per batch: list of global chunk indices
    for (q0, q1, k0, k1) in batches:
        idxs = []
        for (koff, klen) in _chunks(k0, k1, 128):
            idxs.append(len(kchunk_list))
            kchunk_list.append((koff, klen))
        batch_kchunks.append(idxs)
    nch = len(kchunk_list)

    KT_T = 4  # 512 contraction split in 4 k-tiles of 128
    NQC = N // 512
    NZC = N // 128  # 24 z chunks

    with tile.TileContext(nc) as tc:
        with (
            tc.tile_pool(name="consts", bufs=1) as consts,
            tc.tile_pool(name="feat", bufs=2) as featp,
            tc.tile_pool(name="persist", bufs=1) as pers,
            tc.tile_pool(name="pos", bufs=12) as posp,
            tc.tile_pool(name="expp", bufs=14) as expp,
            tc.tile_pool(name="outp", bufs=3) as outp,
            tc.tile_pool(name="ps_s", bufs=4, space="PSUM") as ps_s,
            tc.tile_pool(name="ps_h", bufs=2, space="PSUM") as ps_h,
            tc.tile_pool(name="ps_p", bufs=2, space="PSUM") as ps_p,
            tc.tile_pool(name="dram", bufs=1, space="DRAM") as dramp,
        ):
            # ---------------- constants ----------------
            ones = consts.tile([1, N], BF16)
            nc.vector.memset(ones, 1.0)
            ident64 = consts.tile([D, D], F32)
            make_identity(nc, ident64)
            ident128 = consts.tile([128, 128], BF16)
            make_identity(nc, ident128)

            # warmup collective: pays the CC barrier/firmware init cost up
            # front, overlapped with the projection phase
            ccw_in = dramp.tile([1, 8], F32, tag="ccwi")
            ccw_out = dramp.tile([1, 64], F32, tag="ccwo")
            nc.vector.memset(ccw_sb := consts.tile([1, 8], F32, name="ccw_sb"), 0.0)
            nc.gpsimd.dma_start(out=ccw_in[:, :], in_=ccw_sb[:, :])
            nc.gpsimd.collective_compute(
                "AllGather",
                mybir.AluOpType.bypass,
                replica_groups=[list(range(NCORES))],
                ins=[ccw_in.opt()],
                outs=[ccw_out.opt()],
            )

            wq_sb = consts.tile([128, KT_T, D], BF16)
            wk_sb = consts.tile([128, KT_T, D], BF16)
            wv_sb = consts.tile([128, KT_T, D], BF16)
            woc_sb = consts.tile([128, KT_T, D], BF16)
            for t_d, t_sb in ((wq_d, wq_sb), (wk_d, wk_sb), (wv_d, wv_sb),
                              (woc_d, woc_sb)):
                nc.sync.dma_start(out=t_sb,
                                  in_=t_d.ap().rearrange("(t p) d -> p t d", p=128))
            bias_sb = {}
            for nm, dd in (("bq", bq_d), ("bk", bk_d), ("bv", bv_d)):
                if dd is not None:
                    t = consts.tile([1, D], BF16, tag=f"bias_{nm}")
                    nc.sync.dma_start(out=t, in_=dd[:, :])
                    bias_sb[nm] = t
            if boc_d is not None:
                boc_sb = consts.tile([D, 1], F32)
                nc.sync.dma_start(out=boc_sb, in_=boc_d[:, :])

            # persistent intermediates
            QT_sb = pers.tile([D, N], BF16)    # Q^T/8 with bias folded
            KT_sb = pers.tile([D, N], BF16)
            VT_sb = pers.tile([D, N], F32)
            V_sb = pers.tile([128, nch, D + 1], BF16)  # [k, chunk, d | ones]
            SLICE = 1536
            NSL = N // SLICE
            # per-q-slice tiles so slice post-processing only depends on the
            # batches that actually wrote that slice (Tile dep granularity)
            hT_s = [pers.tile([D, SLICE], BF16, tag=f"hT{s}", name=f"hT{s}")
                    for s in range(NSL)]
            hTn_s = [pers.tile([D, SLICE], BF16, tag=f"hTn{s}", name=f"hTn{s}")
                     for s in range(NSL)]
            Zrow_s = [pers.tile([1, SLICE], F32, tag=f"Zr{s}", name=f"Zr{s}")
                      for s in range(NSL)]
            zbc_s = [pers.tile([D, SLICE], F32, tag=f"zbc{s}", name=f"zbc{s}")
                     for s in range(NSL)]

            # ---------------- projections ----------------
            def project_T(feat_d, w_sb, bias, dst):
                # dst[d, q] = (w.T @ featT)[d, q] (+ bias[d] via rank-1 mm)
                f_sb = featp.tile([128, KT_T, N], BF16, tag="feat")
                for h in range(4):
                    hsl = slice(h * (N // 4), (h + 1) * (N // 4))
                    for t in range(KT_T):
                        eng = nc.scalar if (t % 2 == 0) else nc.gpsimd
                        eng.dma_start(
                            out=f_sb[:, t, hsl],
                            in_=feat_d.ap().rearrange("(t p) n -> t p n", p=128)[t, :, hsl],
                        )
                for qc in range(NQC):
                    qsl = slice(qc * 512, (qc + 1) * 512)
                    psum = ps_p.tile([128, 512], F32, tag="psp")
                    for t in range(KT_T):
                        nc.tensor.matmul(psum[0:D, :], w_sb[:, t, :],
                                         f_sb[:, t, qsl],
                                         start=(t == 0),
                                         stop=(t == KT_T - 1 and bias is None))
                    if bias is not None:
                        nc.tensor.matmul(psum[0:D, :], bias, ones[:, qsl],
                                         start=False, stop=True)
                    nc.vector.tensor_copy(dst[:, qsl], psum[0:D, :])

            project_T(qfT_d, wq_sb, bias_sb.get("bq"), QT_sb)
            project_T(kfT_d, wk_sb, bias_sb.get("bk"), KT_sb)
            project_T(vfT_d, wv_sb, bias_sb.get("bv"), VT_sb)

            # V into batch-aligned k-chunks ([k, d] layout) via PE transposes
            nc.vector.memset(V_sb[:, :, D], 1.0)
            for j, (koff, klen) in enumerate(kchunk_list):
                pst = ps_p.tile([128, 512], F32, tag="psp")
                nc.tensor.transpose(pst[0:klen, 0:D], VT_sb[:, koff:koff + klen],
                                    ident64[:, :])
                if j % 2 == 0:
                    nc.vector.tensor_copy(V_sb[0:klen, j, 0:D], pst[0:klen, 0:D])
                else:
                    nc.scalar.copy(V_sb[0:klen, j, 0:D], pst[0:klen, 0:D])

            # ------------- post-attention pipeline, per q-slice -------------
            def emit_slice(s):
                lo = s * SLICE
                # reciprocal of the Z row in place (~51 ULP approx is far
                # more accurate than needed), bounce through DRAM once, then
                # broadcast-read across the 64 d-partitions.
                zrr = pers.tile([1, SLICE], F32, tag=f"zrr{s}", name=f"zrr{s}")
                nc.vector.reciprocal_approx_fast(zrr[0:1, :], Zrow_s[s][0:1, :])
                zr_d = dramp.tile([1, SLICE], F32, tag=f"zrd{s}")
                nc.gpsimd.dma_start(out=zr_d[:, :], in_=zrr[0:1, :])
                zr_ap = zr_d[:, :]
                zbc_src = bass.AP(tensor=zr_ap.tensor, offset=zr_ap.offset,
                                  ap=[[0, D], [1, SLICE]])
                nc.gpsimd.dma_start(out=zbc_s[s][:, :], in_=zbc_src)
                # normalize hT (gpsimd keeps it off the busy DVE stream)
                nc.gpsimd.tensor_mul(hTn_s[s][:, :], hT_s[s][:, :],
                                     zbc_s[s][:, :])
                # AllGather this slice of hT over heads
                ag_in = dramp.tile([D, SLICE], BF16, tag=f"agi{s}")
                nc.gpsimd.dma_start(out=ag_in[:, :], in_=hTn_s[s][:, :])
                ag_out = dramp.tile([OD, SLICE], BF16, tag=f"ago{s}")
                nc.gpsimd.collective_compute(
                    "AllGather",
                    mybir.AluOpType.bypass,
                    replica_groups=[list(range(NCORES))],
                    ins=[ag_in.opt()],
                    outs=[ag_out.opt()],
                )
                hTf_sb = featp.tile([128, KT_T, SLICE], BF16, tag="feat")
                for t in range(KT_T):
                    nc.gpsimd.dma_start(
                        out=hTf_sb[:, t, :],
                        in_=ag_out[:, :].rearrange("(t p) n -> t p n", p=128)[t],
                    )
                # output projection (column slice of Wo), transposed orientation
                for qc in range(SLICE // 512):
                    osl = slice(lo + qc * 512, lo + (qc + 1) * 512)
                    psum = ps_p.tile([128, 512], F32, tag="psp")
                    for t in range(KT_T):
                        nc.tensor.matmul(psum[0:D, :], woc_sb[:, t, :],
                                         hTf_sb[:, t, qc * 512:(qc + 1) * 512],
                                         start=(t == 0), stop=(t == KT_T - 1))
                    o_sb = outp.tile([D, 512], F32, tag="osb")
                    if boc_d is not None:
                        nc.scalar.activation(o_sb[:, :], psum[0:D, :],
                                             mybir.ActivationFunctionType.Identity,
                                             bias=boc_sb[:, 0:1])
                    else:
                        nc.vector.tensor_copy(o_sb[:, :], psum[0:D, :])
                    nc.gpsimd.dma_start(out=out_d[:, osl], in_=o_sb[:, :])

            # ---------------- attention (block-diagonal) ----------------
            # software-pipelined: unit i's scores/pos/exp are emitted before
            # unit i-1's h-matmuls so the PE never stalls on the DVE->ACT
            # round-trip of the current unit.
            units = [(bi, qoff, qw)
                     for bi, (q0, q1, k0, k1) in enumerate(batches)
                     for (qoff, qw) in _chunks(q0, q1, 512)]
            expts = {}
            emitted = 0

            def stage1(i):
                bi, qoff, qw = units[i]
                qsl = slice(qoff, qoff + qw)
                lst = []
                for j in batch_kchunks[bi]:
                    koff, klen = kchunk_list[j]
                    ksl = slice(koff, koff + klen)
                    ps = ps_s.tile([128, 512], F32, tag="pss")
                    nc.tensor.matmul(ps[0:klen, 0:qw], KT_sb[:, ksl],
                                     QT_sb[:, qsl], start=True, stop=True)
                    pos = posp.tile([128, 512], BF16, tag="pos")
                    nc.sync.dma_start(out=pos[0:klen, 0:qw],
                                      in_=posT_d[ksl, qsl])
                    nc.vector.tensor_add(ps[0:klen, 0:qw], ps[0:klen, 0:qw],
                                         pos[0:klen, 0:qw])
                    expt = expp.tile([128, 512], BF16, tag="expt")
                    nc.scalar.activation(expt[0:klen, 0:qw], ps[0:klen, 0:qw],
                                         mybir.ActivationFunctionType.Exp)
                    lst.append(expt)
                expts[i] = lst

            def stage2(i):
                nonlocal emitted
                bi, qoff, qw = units[i]
                idxs = batch_kchunks[bi]
                psum_h = ps_h.tile([D + 1, 512], F32, tag="psh")
                for ii, j in enumerate(idxs):
                    koff, klen = kchunk_list[j]
                    nc.tensor.matmul(psum_h[:, 0:qw], V_sb[0:klen, j, :],
                                     expts[i][ii][0:klen, 0:qw],
                                     start=(ii == 0), stop=(ii == len(idxs) - 1))
                del expts[i]
                # copy h/Z out of PSUM, split at q-slice boundaries
                seg = qoff
                while seg < qoff + qw:
                    s = seg // SLICE
                    send = min(qoff + qw, (s + 1) * SLICE)
                    lsl = slice(seg - s * SLICE, send - s * SLICE)
                    psl = slice(seg - qoff, send - qoff)
                    nc.vector.tensor_copy(hT_s[s][:, lsl], psum_h[0:D, psl])
                    nc.vector.tensor_copy(Zrow_s[s][:, lsl],
                                          psum_h[D:D + 1, psl])
                    seg = send
                qend = qoff + qw
                while emitted < NSL and (emitted + 1) * SLICE <= qend:
                    emit_slice(emitted)
                    emitted += 1

            LOOK = 2
            for i in range(len(units)):
                stage1(i)
                if i >= LOOK:
                    stage2(i - LOOK)
            for i in range(len(units) - LOOK, len(units)):
                stage2(i)
            assert emitted == NSL, (emitted, NSL, batches)

    nc.compile()
    return nc


def _kernel_numpy(q_feat, k_feat, v_feat, pos_enc, Wq, bq, Wk, bk, Wv, bv,
                  Wo, bo, q_batch, k_batch):
    """Host fallback (degenerate batch layouts only) + debugging aid."""
    Q = (q_feat @ Wq + bq).reshape(N, H, D).transpose(1, 0, 2)
    K = (k_feat @ Wk + bk).reshape(N, H, D).transpose(1, 0, 2)
    V = (v_feat @ Wv + bv).reshape(N, H, D).transpose(1, 0, 2)
    scores = np.einsum("hnd,hmd->hnm", Q, K) / SCALE + pos_enc
    mask = q_batch[:, None] != k_batch[None, :]
    scores = np.where(mask[None], np.float32(-1e9), scores)
    scores = scores - scores.max(-1, keepdims=True)
    e = np.exp(scores)
    probs = e / e.sum(-1, keepdims=True)
    h = np.einsum("hnm,hmd->hnd", probs, V)
    h = h.transpose(1, 0, 2).reshape(N, OD)
    return (h @ Wo + bo).astype(np.float32)


def kernel(q_feat, k_feat, v_feat, pos_enc, Wq, bq, Wk, bk, Wv, bv, Wo, bo,
           q_batch, k_batch):
    global LAST_RESULTS
    args = dict(q_feat=np.asarray(q_feat, np.float32),
                k_feat=np.asarray(k_feat, np.float32),
                v_feat=np.asarray(v_feat, np.float32),
                pos_enc=np.asarray(pos_enc, np.float32),
                Wq=np.asarray(Wq, np.float32), bq=np.asarray(bq, np.float32),
                Wk=np.asarray(Wk, np.float32), bk=np.asarray(bk, np.float32),
                Wv=np.asarray(Wv, np.float32), bv=np.asarray(bv, np.float32),
                Wo=np.asarray(Wo, np.float32), bo=np.asarray(bo, np.float32),
                q_batch=np.asarray(q_batch), k_batch=np.asarray(k_batch))

    batches, degenerate = _plan(args["q_batch"], args["k_batch"])
    if degenerate or not batches:
        return _kernel_numpy(**args)

    has_bq = bool(np.any(args["bq"]))
    has_bk = bool(np.any(args["bk"]))
    has_bv = bool(np.any(args["bv"]))
    has_bo = bool(np.any(args["bo"]))

    nc = _build(batches, has_bq, has_bk, has_bv, has_bo)

    # ---- host-side sharding / layout prep ----
    qfT = np.ascontiguousarray(args["q_feat"].T).astype(BF16_NP)
    kfT = np.ascontiguousarray(args["k_feat"].T).astype(BF16_NP)
    vfT = np.ascontiguousarray(args["v_feat"].T).astype(BF16_NP)

    in_maps = []
    for c in range(NCORES):
        hs = slice(c * D, (c + 1) * D)
        m = {
            "qfT": qfT, "kfT": kfT, "vfT": vfT,
            "posT": np.ascontiguousarray(
                args["pos_enc"][c].astype(BF16_NP).T),
            "wq": (args["Wq"][:, hs] / SCALE).astype(BF16_NP),
            "wk": args["Wk"][:, hs].astype(BF16_NP),
            "wv": args["Wv"][:, hs].astype(BF16_NP),
            "woc": np.ascontiguousarray(args["Wo"][:, hs]).astype(BF16_NP),
        }
        if has_bq:
            m["bq"] = (args["bq"][hs] / SCALE).astype(BF16_NP).reshape(1, D)
        if has_bk:
            m["bk"] = args["bk"][hs].astype(BF16_NP).reshape(1, D)
        if has_bv:
            m["bv"] = args["bv"][hs].astype(BF16_NP).reshape(1, D)
        if has_bo:
            m["boc"] = args["bo"][hs].astype(np.float32).reshape(D, 1)
        in_maps.append(m)

    res = run_bass_kernel_spmd(nc, in_maps, core_ids=list(range(NCORES)),
                               trace=TRACE)
    LAST_RESULTS = res
    out = np.empty((N, OD), np.float32)
    for c in range(NCORES):
        out[:, c * D:(c + 1) * D] = res.results[c]["out"].T
    return out


# revision 33
# speedup vs baseline: 1.5593x; 1.5593x over previous
"""Trainium2 Bass kernel for nn_MultiHeadAttention_3796751090171 (sparse_attention).

Head-parallel SPMD across 8 NeuronCores: core c computes head c's attention,
then the cores AllGather the (normalized) per-head context vectors and each
core computes a 64-column slice of the output projection — so no cross-core
reduction is ever needed (output = pure concatenation).

Math per head h (core c = h):
  Q = q_feat @ Wq[:, h*64:(h+1)*64] + bq_h          [N, 64]   (pre-scaled 1/8)
  K = k_feat @ Wk_h + bk_h                           [N, 64]
  V = v_feat @ Wv_h + bv_h                           [N, 64]
  S = Q @ K.T + pos_enc[h]   (block-diagonal only)   sparse [N, N]
  P = exp(S); Z = row sums (via ones-column in the V matmul)
  hT = (V|1).T @ expT ; hT /= Z
  -- AllGather hT over heads -> hTf [512, N] --
  outT_c = Wo[:, c*64:(c+1)*64].T @ hTf + bo_c       [64, N]
Host: out[:, c*64:(c+1)*64] = outT_c.T

Sparsity: q_batch/k_batch are SORTED, so the cross-batch mask is block-diagonal
over (q-range x k-range) batch blocks; we only compute those blocks and never
need elementwise masking (k-chunks are batch-aligned).

No max-subtraction in softmax: scores are O(10) so exp is safe in fp32; fully
masked blocks are simply never computed (prob contribution exactly 0, matching
the reference's exp(-1e9 - max) -> 0).
"""

import functools
import math

import numpy as np
import ml_dtypes

import concourse.bass as bass
import concourse.tile as tile
from concourse import bacc, mybir
from concourse.bass_utils import run_bass_kernel_spmd
from concourse.masks import make_identity

N = 3072
QD = 512
OD = 512
H = 8
D = 64
B = 8
NCORES = 8
SCALE = math.sqrt(D)

F32 = mybir.dt.float32
BF16 = mybir.dt.bfloat16
BF16_NP = ml_dtypes.bfloat16

# test.py can flip these to get a profile
TRACE = False
LAST_RESULTS = None


def _plan(q_batch, k_batch):
    """Batch block boundaries from the sorted batch-id vectors."""
    qb = np.asarray(q_batch).astype(np.int64)
    kb = np.asarray(k_batch).astype(np.int64)
    qbound = np.searchsorted(qb, np.arange(B + 1))
    kbound = np.searchsorted(kb, np.arange(B + 1))
    batches = []
    degenerate = False
    for b in range(B):
        q0, q1 = int(qbound[b]), int(qbound[b + 1])
        k0, k1 = int(kbound[b]), int(kbound[b + 1])
        if q1 > q0 and k1 > k0:
            batches.append((q0, q1, k0, k1))
        elif q1 > q0 and k1 == k0:
            # rows with no visible keys: reference gives uniform attention over
            # ALL keys; handled by numpy fallback (never happens in practice)
            degenerate = True
    return tuple(batches), degenerate


def _chunks(lo, hi, step):
    return [(o, min(step, hi - o)) for o in range(lo, hi, step)]


@functools.lru_cache(maxsize=8)
def _build(batches, has_bq, has_bk, has_bv, has_bo):
    nc = bacc.Bacc("TRN2", target_bir_lowering=False, debug=False,
                   num_devices=NCORES)

    # ---- DRAM parameters (per-core values supplied via in_maps) ----
    qfT_d = nc.dram_tensor("qfT", [QD, N], BF16, kind="ExternalInput")
    kfT_d = nc.dram_tensor("kfT", [QD, N], BF16, kind="ExternalInput")
    vfT_d = nc.dram_tensor("vfT", [QD, N], BF16, kind="ExternalInput")
    posT_d = nc.dram_tensor("posT", [N, N], BF16, kind="ExternalInput")
    wq_d = nc.dram_tensor("wq", [QD, D], BF16, kind="ExternalInput")
    wk_d = nc.dram_tensor("wk", [QD, D], BF16, kind="ExternalInput")
    wv_d = nc.dram_tensor("wv", [QD, D], BF16, kind="ExternalInput")
    woc_d = nc.dram_tensor("woc", [OD, D], BF16, kind="ExternalInput")
    bq_d = nc.dram_tensor("bq", [1, D], BF16, kind="ExternalInput") if has_bq else None
    bk_d = nc.dram_tensor("bk", [1, D], BF16, kind="ExternalInput") if has_bk else None
    bv_d = nc.dram_tensor("bv", [1, D], BF16, kind="ExternalInput") if has_bv else None
    boc_d = nc.dram_tensor("boc", [D, 1], F32, kind="ExternalInput") if has_bo else None
    out_d = nc.dram_tensor("out", [D, N], F32, kind="ExternalOutput")

    # global k-chunk list, aligned to batch boundaries (never crosses one)
    kchunk_list = []   # (koff, klen)
    batch_kchunks = []  # per batch: list of global chunk indices
    for (q0, q1, k0, k1) in batches:
        idxs = []
        for (koff, klen) in _chunks(k0, k1, 128):
            idxs.append(len(kchunk_list))
            kchunk_list.append((koff, klen))
        batch_kchunks.append(idxs)
    nch = len(kchunk_list)

    KT_T = 4  # 512 contraction split in 4 k-tiles of 128
    NQC = N // 512
    NZC = N // 128  # 24 z chunks

    with tile.TileContext(nc) as tc:
        with (
            tc.tile_pool(name="consts", bufs=1) as consts,
            tc.tile_pool(name="feat", bufs=2) as featp,
            tc.tile_pool(name="persist", bufs=1) as pers,
            tc.tile_pool(name="pos", bufs=12) as posp,
            tc.tile_pool(name="expp", bufs=14) as expp,
            tc.tile_pool(name="outp", bufs=3) as outp,
            tc.tile_pool(name="ps_s", bufs=4, space="PSUM") as ps_s,
            tc.tile_pool(name="ps_h", bufs=2, space="PSUM") as ps_h,
            tc.tile_pool(name="ps_p", bufs=2, space="PSUM") as ps_p,
            tc.tile_pool(name="dram", bufs=1, space="DRAM") as dramp,
        ):
            # ---------------- constants ----------------
            ones = consts.tile([1, N], BF16)
            nc.vector.memset(ones, 1.0)
            ident64 = consts.tile([D, D], F32)
            make_identity(nc, ident64)
            ident128 = consts.tile([128, 128], BF16)
            make_identity(nc, ident128)

            # warmup collective: pays the CC barrier/firmware init cost up
            # front, overlapped with the projection phase
            ccw_in = dramp.tile([1, 8], F32, tag="ccwi")
            ccw_out = dramp.tile([1, 64], F32, tag="ccwo")
            nc.vector.memset(ccw_sb := consts.tile([1, 8], F32, name="ccw_sb"), 0.0)
            nc.gpsimd.dma_start(out=ccw_in[:, :], in_=ccw_sb[:, :])
            nc.gpsimd.collective_compute(
                "AllGather",
                mybir.AluOpType.bypass,
                replica_groups=[list(range(NCORES))],
                ins=[ccw_in.opt()],
                outs=[ccw_out.opt()],
            )

            wq_sb = consts.tile([128, KT_T, D], BF16)
            wk_sb = consts.tile([128, KT_T, D], BF16)
            wv_sb = consts.tile([128, KT_T, D], BF16)
            woc_sb = consts.tile([128, KT_T, D], BF16)
            for t_d, t_sb in ((wq_d, wq_sb), (wk_d, wk_sb), (wv_d, wv_sb),
                              (woc_d, woc_sb)):
                nc.sync.dma_start(out=t_sb,
                                  in_=t_d.ap().rearrange("(t p) d -> p t d", p=128))
            bias_sb = {}
            for nm, dd in (("bq", bq_d), ("bk", bk_d), ("bv", bv_d)):
                if dd is not None:
                    t = consts.tile([1, D], BF16, tag=f"bias_{nm}")
                    nc.sync.dma_start(out=t, in_=dd[:, :])
                    bias_sb[nm] = t
            if boc_d is not None:
                boc_sb = consts.tile([D, 1], F32)
                nc.sync.dma_start(out=boc_sb, in_=boc_d[:, :])

            # persistent intermediates
            QT_sb = pers.tile([D, N], BF16)    # Q^T/8 with bias folded
            KT_sb = pers.tile([D, N], BF16)
            VT_sb = pers.tile([D, N], F32)
            V_sb = pers.tile([128, nch, D + 1], BF16)  # [k, chunk, d | ones]
            SLICE = 1536
            NSL = N // SLICE
            # per-q-slice tiles so slice post-processing only depends on the
            # batches that actually wrote that slice (Tile dep granularity)
            hT_s = [pers.tile([D, SLICE], BF16, tag=f"hT{s}", name=f"hT{s}")
                    for s in range(NSL)]
            hTn_s = [pers.tile([D, SLICE], BF16, tag=f"hTn{s}", name=f"hTn{s}")
                     for s in range(NSL)]
            Zrow_s = [pers.tile([1, SLICE], F32, tag=f"Zr{s}", name=f"Zr{s}")
                      for s in range(NSL)]
            zbc_s = [pers.tile([D, SLICE], F32, tag=f"zbc{s}", name=f"zbc{s}")
                     for s in range(NSL)]

            # ---------------- projections ----------------
            def project_T(feat_d, w_sb, bias, dst):
                # dst[d, q] = (w.T @ featT)[d, q] (+ bias[d] via rank-1 mm)
                f_sb = featp.tile([128, KT_T, N], BF16, tag="feat")
                for h in range(4):
                    hsl = slice(h * (N // 4), (h + 1) * (N // 4))
                    for t in range(KT_T):
                        nc.gpsimd.dma_start(
                            out=f_sb[:, t, hsl],
                            in_=feat_d.ap().rearrange("(t p) n -> t p n", p=128)[t, :, hsl],
                        )
                for qc in range(NQC):
                    qsl = slice(qc * 512, (qc + 1) * 512)
                    psum = ps_p.tile([128, 512], F32, tag="psp")
                    for t in range(KT_T):
                        nc.tensor.matmul(psum[0:D, :], w_sb[:, t, :],
                                         f_sb[:, t, qsl],
                                         start=(t == 0),
                                         stop=(t == KT_T - 1 and bias is None))
                    if bias is not None:
                        nc.tensor.matmul(psum[0:D, :], bias, ones[:, qsl],
                                         start=False, stop=True)
                    nc.vector.tensor_copy(dst[:, qsl], psum[0:D, :])

            project_T(qfT_d, wq_sb, bias_sb.get("bq"), QT_sb)
            project_T(kfT_d, wk_sb, bias_sb.get("bk"), KT_sb)
            project_T(vfT_d, wv_sb, bias_sb.get("bv"), VT_sb)

            # V into batch-aligned k-chunks ([k, d] layout) via PE transposes
            nc.vector.memset(V_sb[:, :, D], 1.0)
            for j, (koff, klen) in enumerate(kchunk_list):
                pst = ps_p.tile([128, 512], F32, tag="psp")
                nc.tensor.transpose(pst[0:klen, 0:D], VT_sb[:, koff:koff + klen],
                                    ident64[:, :])
                if j % 2 == 0:
                    nc.vector.tensor_copy(V_sb[0:klen, j, 0:D], pst[0:klen, 0:D])
                else:
                    nc.scalar.copy(V_sb[0:klen, j, 0:D], pst[0:klen, 0:D])

            # ------------- post-attention pipeline, per q-slice -------------
            def emit_slice(s):
                lo = s * SLICE
                # reciprocal of the Z row in place (~51 ULP approx is far
                # more accurate than needed), bounce through DRAM once, then
                # broadcast-read across the 64 d-partitions.
                zrr = pers.tile([1, SLICE], F32, tag=f"zrr{s}", name=f"zrr{s}")
                nc.vector.reciprocal_approx_fast(zrr[0:1, :], Zrow_s[s][0:1, :])
                zr_d = dramp.tile([1, SLICE], F32, tag=f"zrd{s}")
                nc.gpsimd.dma_start(out=zr_d[:, :], in_=zrr[0:1, :])
                zr_ap = zr_d[:, :]
                zbc_src = bass.AP(tensor=zr_ap.tensor, offset=zr_ap.offset,
                                  ap=[[0, D], [1, SLICE]])
                nc.gpsimd.dma_start(out=zbc_s[s][:, :], in_=zbc_src)
                # normalize hT (gpsimd keeps it off the busy DVE stream)
                nc.gpsimd.tensor_mul(hTn_s[s][:, :], hT_s[s][:, :],
                                     zbc_s[s][:, :])
                # AllGather this slice of hT over heads
                ag_in = dramp.tile([D, SLICE], BF16, tag=f"agi{s}")
                nc.gpsimd.dma_start(out=ag_in[:, :], in_=hTn_s[s][:, :])
                ag_out = dramp.tile([OD, SLICE], BF16, tag=f"ago{s}")
                nc.gpsimd.collective_compute(
                    "AllGather",
                    mybir.AluOpType.bypass,
                    replica_groups=[list(range(NCORES))],
                    ins=[ag_in.opt()],
                    outs=[ag_out.opt()],
                )
                hTf_sb = featp.tile([128, KT_T, SLICE], BF16, tag="feat")
                for t in range(KT_T):
                    nc.gpsimd.dma_start(
                        out=hTf_sb[:, t, :],
                        in_=ag_out[:, :].rearrange("(t p) n -> t p n", p=128)[t],
                    )
                # output projection (column slice of Wo), transposed orientation
                for qc in range(SLICE // 512):
                    osl = slice(lo + qc * 512, lo + (qc + 1) * 512)
                    psum = ps_p.tile([128, 512], F32, tag="psp")
                    for t in range(KT_T):
                        nc.tensor.matmul(psum[0:D, :], woc_sb[:, t, :],
                                         hTf_sb[:, t, qc * 512:(qc + 1) * 512],
                                         start=(t == 0), stop=(t == KT_T - 1))
                    o_sb = outp.tile([D, 512], F32, tag="osb")
                    if boc_d is not None:
                        nc.scalar.activation(o_sb[:, :], psum[0:D, :],
                                             mybir.ActivationFunctionType.Identity,
                                             bias=boc_sb[:, 0:1])
                    else:
                        nc.vector.tensor_copy(o_sb[:, :], psum[0:D, :])
                    nc.gpsimd.dma_start(out=out_d[:, osl], in_=o_sb[:, :])

            # ---------------- attention (block-diagonal) ----------------
            # software-pipelined: unit i's scores/pos/exp are emitted before
            # unit i-1's h-matmuls so the PE never stalls on the DVE->ACT
            # round-trip of the current unit.
            units = [(bi, qoff, qw)
                     for bi, (q0, q1, k0, k1) in enumerate(batches)
                     for (qoff, qw) in _chunks(q0, q1, 512)]
            expts = {}
            emitted = 0

            def stage1(i):
                bi, qoff, qw = units[i]
                qsl = slice(qoff, qoff + qw)
                lst = []
                for j in batch_kchunks[bi]:
                    koff, klen = kchunk_list[j]
                    ksl = slice(koff, koff + klen)
                    ps = ps_s.tile([128, 512], F32, tag="pss")
                    nc.tensor.matmul(ps[0:klen, 0:qw], KT_sb[:, ksl],
                                     QT_sb[:, qsl], start=True, stop=True)
                    pos = posp.tile([128, 512], BF16, tag="pos")
                    nc.sync.dma_start(out=pos[0:klen, 0:qw],
                                      in_=posT_d[ksl, qsl])
                    nc.vector.tensor_add(ps[0:klen, 0:qw], ps[0:klen, 0:qw],
                                         pos[0:klen, 0:qw])
                    expt = expp.tile([128, 512], BF16, tag="expt")
                    nc.scalar.activation(expt[0:klen, 0:qw], ps[0:klen, 0:qw],
                                         mybir.ActivationFunctionType.Exp)
                    lst.append(expt)
                expts[i] = lst

            def stage2(i):
                nonlocal emitted
                bi, qoff, qw = units[i]
                idxs = batch_kchunks[bi]
                psum_h = ps_h.tile([D + 1, 512], F32, tag="psh")
                for ii, j in enumerate(idxs):
                    koff, klen = kchunk_list[j]
                    nc.tensor.matmul(psum_h[:, 0:qw], V_sb[0:klen, j, :],
                                     expts[i][ii][0:klen, 0:qw],
                                     start=(ii == 0), stop=(ii == len(idxs) - 1))
                del expts[i]
                # copy h/Z out of PSUM, split at q-slice boundaries
                seg = qoff
                while seg < qoff + qw:
                    s = seg // SLICE
                    send = min(qoff + qw, (s + 1) * SLICE)
                    lsl = slice(seg - s * SLICE, send - s * SLICE)
                    psl = slice(seg - qoff, send - qoff)
                    nc.vector.tensor_copy(hT_s[s][:, lsl], psum_h[0:D, psl])
                    nc.vector.tensor_copy(Zrow_s[s][:, lsl],
                                          psum_h[D:D + 1, psl])
                    seg = send
                qend = qoff + qw
                while emitted < NSL and (emitted + 1) * SLICE <= qend:
                    emit_slice(emitted)
                    emitted += 1

            LOOK = 2
            for i in range(len(units)):
                stage1(i)
                if i >= LOOK:
                    stage2(i - LOOK)
            for i in range(len(units) - LOOK, len(units)):
                stage2(i)
            assert emitted == NSL, (emitted, NSL, batches)

    nc.compile()
    return nc


def _kernel_numpy(q_feat, k_feat, v_feat, pos_enc, Wq, bq, Wk, bk, Wv, bv,
                  Wo, bo, q_batch, k_batch):
    """Host fallback (degenerate batch layouts only) + debugging aid."""
    Q = (q_feat @ Wq + bq).reshape(N, H, D).transpose(1, 0, 2)
    K = (k_feat @ Wk + bk).reshape(N, H, D).transpose(1, 0, 2)
    V = (v_feat @ Wv + bv).reshape(N, H, D).transpose(1, 0, 2)
    scores = np.einsum("hnd,hmd->hnm", Q, K) / SCALE + pos_enc
    mask = q_batch[:, None] != k_batch[None, :]
    scores = np.where(mask[None], np.float32(-1e9), scores)
    scores = scores - scores.max(-1, keepdims=True)
    e = np.exp(scores)
    probs = e / e.sum(-1, keepdims=True)
    h = np.einsum("hnm,hmd->hnd", probs, V)
    h = h.transpose(1, 0, 2).reshape(N, OD)
    return (h @ Wo + bo).astype(np.float32)


def kernel(q_feat, k_feat, v_feat, pos_enc, Wq, bq, Wk, bk, Wv, bv, Wo, bo,
           q_batch, k_batch):
    global LAST_RESULTS
    args = dict(q_feat=np.asarray(q_feat, np.float32),
                k_feat=np.asarray(k_feat, np.float32),
                v_feat=np.asarray(v_feat, np.float32),
                pos_enc=np.asarray(pos_enc, np.float32),
                Wq=np.asarray(Wq, np.float32), bq=np.asarray(bq, np.float32),
                Wk=np.asarray(Wk, np.float32), bk=np.asarray(bk, np.float32),
                Wv=np.asarray(Wv, np.float32), bv=np.asarray(bv, np.float32),
                Wo=np.asarray(Wo, np.float32), bo=np.asarray(bo, np.float32),
                q_batch=np.asarray(q_batch), k_batch=np.asarray(k_batch))

    batches, degenerate = _plan(args["q_batch"], args["k_batch"])
    if degenerate or not batches:
        return _kernel_numpy(**args)

    has_bq = bool(np.any(args["bq"]))
    has_bk = bool(np.any(args["bk"]))
    has_bv = bool(np.any(args["bv"]))
    has_bo = bool(np.any(args["bo"]))

    nc = _build(batches, has_bq, has_bk, has_bv, has_bo)

    # ---- host-side sharding / layout prep ----
    qfT = np.ascontiguousarray(args["q_feat"].T).astype(BF16_NP)
    kfT = np.ascontiguousarray(args["k_feat"].T).astype(BF16_NP)
    vfT = np.ascontiguousarray(args["v_feat"].T).astype(BF16_NP)

    in_maps = []
    for c in range(NCORES):
        hs = slice(c * D, (c + 1) * D)
        m = {
            "qfT": qfT, "kfT": kfT, "vfT": vfT,
            "posT": np.ascontiguousarray(
                args["pos_enc"][c].astype(BF16_NP).T),
            "wq": (args["Wq"][:, hs] / SCALE).astype(BF16_NP),
            "wk": args["Wk"][:, hs].astype(BF16_NP),
            "wv": args["Wv"][:, hs].astype(BF16_NP),
            "woc": np.ascontiguousarray(args["Wo"][:, hs]).astype(BF16_NP),
        }
        if has_bq:
            m["bq"] = (args["bq"][hs] / SCALE).astype(BF16_NP).reshape(1, D)
        if has_bk:
            m["bk"] = args["bk"][hs].astype(BF16_NP).reshape(1, D)
        if has_bv:
            m["bv"] = args["bv"][hs].astype(BF16_NP).reshape(1, D)
        if has_bo:
            m["boc"] = args["bo"][hs].astype(np.float32).reshape(D, 1)
        in_maps.append(m)

    res = run_bass_kernel_spmd(nc, in_maps, core_ids=list(range(NCORES)),
                               trace=TRACE)
    LAST_RESULTS = res
    out = np.empty((N, OD), np.float32)
    for c in range(NCORES):
        out[:, c * D:(c + 1) * D] = res.results[c]["out"].T
    return out


# revision 36
# speedup vs baseline: 2.4036x; 1.5415x over previous
"""Trainium2 Bass kernel for nn_MultiHeadAttention_3796751090171 (sparse_attention).

Batch-parallel SPMD across 8 NeuronCores: q_batch/k_batch are SORTED, so the
cross-batch mask makes attention block-diagonal over batches, and there are
exactly B=8 batches for 8 cores. Core c computes batch c's queries against
batch c's keys for ALL 8 heads -- completely independent work, so there are
NO collectives: the full output is a pure row-concatenation of the per-core
outputs.

Uniform SPMD template: every core runs the same program on [NKMAX x NQMAX]
padded tiles (NQMAX/NKMAX = max batch size rounded up to 128). The host pads
each core's feature slices with zeros and fabricates the pos_enc tile so that
  - padded k-rows carry pos = -1e9  -> exp = 0, no contribution to h or Z
  - padded q-cols carry pos = 0 on real k-rows (keeps Z finite; outputs for
    those columns are dropped by the host)

Per core c (batch slice qs:qe / ks:ke, all heads h):
  Q = qf[qs:qe] @ Wq/8, K = kf[ks:ke] @ Wk, V = vf[ks:ke] @ Wv  (+biases)
  per head: scoresT[k,q] = K_h^T-chunks @ Q_h + posT  (PSUM)
  expT = exp(scoresT); hT_unnorm/Z via [V|1] matmul (ones column -> row 64 = Z)
  hTn = hT * (1/Z broadcast); out^T[o,q] = sum_h Wo[64h:,:].T @ hTn_h + bo
Host: out[qs:qe, :] = outT[:, :nq].T

No max-subtraction in softmax: scores are O(10) so exp is safe in fp32; masked
entries give exp(-1e9+...) -> exactly 0, matching the reference's
exp(-1e9 - max) -> 0.
"""

import functools
import math

import numpy as np
import ml_dtypes

import concourse.bass as bass
import concourse.tile as tile
from concourse import bacc, mybir
from concourse.bass_utils import run_bass_kernel_spmd
from concourse.masks import make_identity

N = 3072
QD = 512
OD = 512
H = 8
D = 64
B = 8
NCORES = 8
SCALE = math.sqrt(D)

F32 = mybir.dt.float32
BF16 = mybir.dt.bfloat16
BF16_NP = ml_dtypes.bfloat16

TRACE = False
LAST_RESULTS = None


def _bounds(q_batch, k_batch):
    qb = np.asarray(q_batch).astype(np.int64)
    kb = np.asarray(k_batch).astype(np.int64)
    qbound = np.searchsorted(qb, np.arange(B + 1))
    kbound = np.searchsorted(kb, np.arange(B + 1))
    return qbound, kbound


def _chunks(lo, hi, step):
    return [(o, min(step, hi - o)) for o in range(lo, hi, step)]


def _r128(x):
    return max(128, ((x + 127) // 128) * 128)


@functools.lru_cache(maxsize=8)
def _build(NQ, NK, has_bq, has_bk, has_bv, has_bo):
    nc = bacc.Bacc("TRN2", target_bir_lowering=False, debug=False,
                   num_devices=NCORES)

    KT_T = QD // 128   # 4 contraction tiles for the projections
    NKC = NK // 128    # k chunks
    QCH = _chunks(0, NQ, 512)   # q chunks (free-dim <= 512)
    NTD = QD // 128    # output-d tiles for projections

    qfT_d = nc.dram_tensor("qfT", [QD, NQ], BF16, kind="ExternalInput")
    kfT_d = nc.dram_tensor("kfT", [QD, NK], BF16, kind="ExternalInput")
    vfT_d = nc.dram_tensor("vfT", [QD, NK], BF16, kind="ExternalInput")
    posc_d = nc.dram_tensor("posc", [H, NK, NQ], BF16, kind="ExternalInput")
    wq_d = nc.dram_tensor("wq", [QD, OD], BF16, kind="ExternalInput")
    wk_d = nc.dram_tensor("wk", [QD, OD], BF16, kind="ExternalInput")
    wv_d = nc.dram_tensor("wv", [QD, OD], BF16, kind="ExternalInput")
    wo_d = nc.dram_tensor("wo", [OD, OD], BF16, kind="ExternalInput")
    bq_d = nc.dram_tensor("bq", [1, OD], BF16, kind="ExternalInput") if has_bq else None
    bk_d = nc.dram_tensor("bk", [1, OD], BF16, kind="ExternalInput") if has_bk else None
    bv_d = nc.dram_tensor("bv", [1, OD], BF16, kind="ExternalInput") if has_bv else None
    bo_d = nc.dram_tensor("bo", [128, NTD], F32, kind="ExternalInput") if has_bo else None
    out_d = nc.dram_tensor("out", [OD, NQ], F32, kind="ExternalOutput")
    import os
    DEBUG = bool(os.environ.get("KDBG"))
    if DEBUG:
        dbg_h = nc.dram_tensor("dbg_h", [D, H, NQ], F32, kind="ExternalOutput")
        dbg_z = nc.dram_tensor("dbg_z", [1, H, NQ], F32, kind="ExternalOutput")
        dbg_e = nc.dram_tensor("dbg_e", [128, NQ], F32, kind="ExternalOutput")

    with tile.TileContext(nc) as tc:
        with (
            tc.tile_pool(name="consts", bufs=1) as consts,
            tc.tile_pool(name="pos", bufs=12) as posp,
            tc.tile_pool(name="expp", bufs=12) as expp,
            tc.tile_pool(name="outp", bufs=4) as outp,
            tc.tile_pool(name="ps_s", bufs=4, space="PSUM") as ps_s,
            tc.tile_pool(name="ps_h", bufs=2, space="PSUM") as ps_h,
            tc.tile_pool(name="ps_p", bufs=2, space="PSUM") as ps_p,
            tc.tile_pool(name="dram", bufs=1, space="DRAM") as dramp,
        ):
            # ---------------- constants / weights ----------------
            ones = consts.tile([1, max(NQ, NK)], BF16)
            nc.vector.memset(ones, 1.0)
            ident64 = consts.tile([D, D], BF16)
            make_identity(nc, ident64)

            wq_sb = consts.tile([128, KT_T, OD], BF16)
            wk_sb = consts.tile([128, KT_T, OD], BF16)
            wv_sb = consts.tile([128, KT_T, OD], BF16)
            for t_d, t_sb in ((wq_d, wq_sb), (wk_d, wk_sb), (wv_d, wv_sb)):
                nc.gpsimd.dma_start(
                    out=t_sb, in_=t_d.ap().rearrange("(t p) d -> p t d", p=128))
            # wo as [64, h, oc, 128] for contraction-64 output projection
            wo_sb = consts.tile([D, H, NTD, 128], BF16)
            nc.gpsimd.dma_start(
                out=wo_sb,
                in_=wo_d.ap().rearrange("(h p) (o c) -> p h o c", p=D, c=128))
            bias_sb = {}
            for nm, dd in (("bq", bq_d), ("bk", bk_d), ("bv", bv_d)):
                if dd is not None:
                    t = consts.tile([1, OD], BF16, tag=f"bias_{nm}", name=f"b_{nm}")
                    nc.gpsimd.dma_start(out=t, in_=dd[:, :])
                    bias_sb[nm] = t
            if bo_d is not None:
                bo_sb = consts.tile([128, NTD], F32)
                nc.gpsimd.dma_start(out=bo_sb, in_=bo_d[:, :])

            # feature tiles
            qf_sb = consts.tile([128, KT_T, NQ], BF16)
            kf_sb = consts.tile([128, KT_T, NK], BF16)
            vf_sb = consts.tile([128, KT_T, NK], BF16)
            for f_d, f_sb in ((qfT_d, qf_sb), (kfT_d, kf_sb), (vfT_d, vf_sb)):
                for t in range(KT_T):
                    nc.gpsimd.dma_start(
                        out=f_sb[:, t, :],
                        in_=f_d.ap().rearrange("(t p) n -> t p n", p=128)[t])

            # projected tensors, split by head parity so every matmul operand
            # sits at partition base 0 (slot index = head // 2)
            QT_e = consts.tile([D, NTD, NQ], BF16, name="QT_e")
            QT_o = consts.tile([D, NTD, NQ], BF16, name="QT_o")
            KT_e = consts.tile([D, NTD, NK], BF16, name="KT_e")
            KT_o = consts.tile([D, NTD, NK], BF16, name="KT_o")
            VT_e = consts.tile([D, NTD, NK], BF16, name="VT_e")
            VT_o = consts.tile([D, NTD, NK], BF16, name="VT_o")
            V_sb = consts.tile([128, H, NKC, D + 1], BF16, name="V_sb")
            hT_sb = consts.tile([D, H, NQ], BF16, name="hT_sb")
            hTn_sb = consts.tile([D, H, NQ], BF16, name="hTn_sb")
            zall_sb = consts.tile([1, H, NQ], F32, name="zall_sb")
            zrec_sb = consts.tile([1, H, NQ], F32, name="zrec_sb")
            zbc_sb = consts.tile([D, H, NQ], F32, name="zbc_sb")

            # ---------------- projections ----------------
            def project(f_sb, w_sb, bias, dstE, dstO, xchunks):
                for td in range(NTD):
                    dsl = slice(128 * td, 128 * (td + 1))
                    for (xo, xw) in xchunks:
                        xsl = slice(xo, xo + xw)
                        psum = ps_p.tile([128, 512], F32, tag="psp")
                        for t in range(KT_T):
                            nc.tensor.matmul(psum[:, 0:xw],
                                             w_sb[:, t, dsl], f_sb[:, t, xsl],
                                             start=(t == 0),
                                             stop=(t == KT_T - 1 and bias is None))
                        if bias is not None:
                            nc.tensor.matmul(psum[:, 0:xw], bias[:, dsl],
                                             ones[:, xsl], start=False, stop=True)
                        nc.vector.tensor_copy(dstE[:, td, xsl], psum[0:D, 0:xw])
                        nc.scalar.copy(dstO[:, td, xsl], psum[D:128, 0:xw])

            kchunks = _chunks(0, NK, 512)
            project(qf_sb, wq_sb, bias_sb.get("bq"), QT_e, QT_o, QCH)
            project(kf_sb, wk_sb, bias_sb.get("bk"), KT_e, KT_o, kchunks)
            project(vf_sb, wv_sb, bias_sb.get("bv"), VT_e, VT_o, kchunks)

            # V into [k, d | ones] per (head, kchunk) via PE transposes
            nc.vector.memset(V_sb[:, :, :, D], 1.0)
            for h in range(H):
                VT = VT_e if h % 2 == 0 else VT_o
                for kc in range(NKC):
                    ksl = slice(128 * kc, 128 * (kc + 1))
                    pst = ps_p.tile([128, 512], BF16, tag="psp")
                    nc.tensor.transpose(pst[:, 0:D], VT[:, h // 2, ksl],
                                        ident64[:, :])
                    nc.scalar.copy(V_sb[:, h, kc, 0:D], pst[:, 0:D])

            # ---------------- attention, software-pipelined ----------------
            units = [(h, qo, qw) for h in range(H) for (qo, qw) in QCH]
            expts = {}

            def stage1(i):
                h, qo, qw = units[i]
                qsl = slice(qo, qo + qw)
                QT = QT_e if h % 2 == 0 else QT_o
                KT = KT_e if h % 2 == 0 else KT_o
                lst = []
                for kc in range(NKC):
                    ksl = slice(128 * kc, 128 * (kc + 1))
                    ps = ps_s.tile([128, 512], F32, tag="pss")
                    nc.tensor.matmul(ps[:, 0:qw], KT[:, h // 2, ksl],
                                     QT[:, h // 2, qsl], start=True, stop=True)
                    pos = posp.tile([128, 512], BF16, tag="pos")
                    nc.sync.dma_start(out=pos[:, 0:qw], in_=posc_d[h, ksl, qsl])
                    nc.vector.tensor_add(ps[:, 0:qw], ps[:, 0:qw], pos[:, 0:qw])
                    expt = expp.tile([128, 512], BF16, tag="expt")
                    nc.scalar.activation(expt[:, 0:qw], ps[:, 0:qw],
                                         mybir.ActivationFunctionType.Exp)
                    if DEBUG and i == 0 and kc == 0:
                        de = consts.tile([128, NQ], F32, name="de")
                        nc.vector.tensor_copy(de[:, 0:qw], expt[:, 0:qw])
                        nc.gpsimd.dma_start(out=dbg_e.ap(), in_=de[:, :])
                    lst.append(expt)
                expts[i] = lst

            def stage2(i):
                h, qo, qw = units[i]
                qsl = slice(qo, qo + qw)
                psum_h = ps_h.tile([D + 1, 512], F32, tag="psh")
                for kc in range(NKC):
                    nc.tensor.matmul(psum_h[:, 0:qw], V_sb[:, h, kc, :],
                                     expts[i][kc][:, 0:qw],
                                     start=(kc == 0), stop=(kc == NKC - 1))
                del expts[i]
                nc.scalar.copy(hT_sb[:, h, qsl], psum_h[0:D, 0:qw])
                # 1/Z for this unit (Z = row 64 of the accumulator); the
                # approx reciprocal requires an SBUF source, so copy first
                nc.scalar.copy(zall_sb[0:1, h, qsl], psum_h[D:D + 1, 0:qw])
                nc.vector.reciprocal_approx_fast(zrec_sb[0:1, h, qsl],
                                                 zall_sb[0:1, h, qsl])

            LOOK = 2
            for i in range(len(units)):
                stage1(i)
                if i >= LOOK:
                    stage2(i - LOOK)
            for i in range(max(0, len(units) - LOOK), len(units)):
                stage2(i)

            if DEBUG:
                dh = consts.tile([D, H, NQ], F32, name="dh")
                nc.vector.tensor_copy(dh[:, :, :], hT_sb[:, :, :])
                nc.gpsimd.dma_start(out=dbg_h.ap(), in_=dh[:, :, :])
                nc.gpsimd.dma_start(out=dbg_z.ap(), in_=zrec_sb[0:1, :, :])

            # ---------------- normalize + output projection ----------------
            zr_d = dramp.tile([1, H * NQ], F32)
            nc.gpsimd.dma_start(out=zr_d[:, :],
                                in_=zrec_sb[0:1, :, :].rearrange("p h n -> p (h n)"))
            zr_ap = zr_d[:, :]
            zbc_src = bass.AP(tensor=zr_ap.tensor, offset=zr_ap.offset,
                              ap=[[0, D], [1, H * NQ]])
            nc.gpsimd.dma_start(
                out=zbc_sb[:, :, :].rearrange("p h n -> p (h n)"), in_=zbc_src)
            nc.vector.tensor_mul(
                hTn_sb[:, :, :].rearrange("p h n -> p (h n)"),
                hT_sb[:, :, :].rearrange("p h n -> p (h n)"),
                zbc_sb[:, :, :].rearrange("p h n -> p (h n)"))

            for oc in range(NTD):
                for (qo, qw) in QCH:
                    qsl = slice(qo, qo + qw)
                    psum = ps_p.tile([128, 512], F32, tag="psp")
                    for h in range(H):
                        nc.tensor.matmul(psum[:, 0:qw], wo_sb[:, h, oc, :],
                                         hTn_sb[:, h, qsl],
                                         start=(h == 0), stop=(h == H - 1))
                    o_sb = outp.tile([128, 512], F32, tag="osb")
                    if bo_d is not None:
                        nc.scalar.activation(o_sb[:, 0:qw], psum[:, 0:qw],
                                             mybir.ActivationFunctionType.Identity,
                                             bias=bo_sb[:, oc:oc + 1])
                    else:
                        nc.vector.tensor_copy(o_sb[:, 0:qw], psum[:, 0:qw])
                    nc.gpsimd.dma_start(out=out_d[128 * oc:128 * (oc + 1), qsl],
                                        in_=o_sb[:, 0:qw])

    nc.compile()
    return nc


def _kernel_numpy(q_feat, k_feat, v_feat, pos_enc, Wq, bq, Wk, bk, Wv, bv,
                  Wo, bo, q_batch, k_batch):
    """Host fallback (degenerate batch layouts) + debugging aid."""
    Q = (q_feat @ Wq + bq).reshape(N, H, D).transpose(1, 0, 2)
    K = (k_feat @ Wk + bk).reshape(N, H, D).transpose(1, 0, 2)
    V = (v_feat @ Wv + bv).reshape(N, H, D).transpose(1, 0, 2)
    scores = np.einsum("hnd,hmd->hnm", Q, K) / SCALE + pos_enc
    mask = q_batch[:, None] != k_batch[None, :]
    scores = np.where(mask[None], np.float32(-1e9), scores)
    scores = scores - scores.max(-1, keepdims=True)
    e = np.exp(scores)
    probs = e / e.sum(-1, keepdims=True)
    h = np.einsum("hnm,hmd->hnd", probs, V)
    h = h.transpose(1, 0, 2).reshape(N, OD)
    return (h @ Wo + bo).astype(np.float32)


def kernel(q_feat, k_feat, v_feat, pos_enc, Wq, bq, Wk, bk, Wv, bv, Wo, bo,
           q_batch, k_batch):
    global LAST_RESULTS
    args = dict(q_feat=np.asarray(q_feat, np.float32),
                k_feat=np.asarray(k_feat, np.float32),
                v_feat=np.asarray(v_feat, np.float32),
                pos_enc=np.asarray(pos_enc, np.float32),
                Wq=np.asarray(Wq, np.float32), bq=np.asarray(bq, np.float32),
                Wk=np.asarray(Wk, np.float32), bk=np.asarray(bk, np.float32),
                Wv=np.asarray(Wv, np.float32), bv=np.asarray(bv, np.float32),
                Wo=np.asarray(Wo, np.float32), bo=np.asarray(bo, np.float32),
                q_batch=np.asarray(q_batch), k_batch=np.asarray(k_batch))

    qbound, kbound = _bounds(args["q_batch"], args["k_batch"])
    nq_all = np.diff(qbound)
    nk_all = np.diff(kbound)
    if np.any((nq_all > 0) & (nk_all == 0)):
        # some batch has queries but no keys: reference gives uniform attention
        # over ALL keys there; fall back (never happens with realistic inputs)
        return _kernel_numpy(**args)

    NQ = _r128(int(nq_all.max()))
    NK = _r128(int(nk_all.max()))

    has_bq = bool(np.any(args["bq"]))
    has_bk = bool(np.any(args["bk"]))
    has_bv = bool(np.any(args["bv"]))
    has_bo = bool(np.any(args["bo"]))

    nc = _build(NQ, NK, has_bq, has_bk, has_bv, has_bo)

    # ---- host-side sharding / layout / padding ----
    qfT = np.ascontiguousarray(args["q_feat"].T).astype(BF16_NP)
    kfT = np.ascontiguousarray(args["k_feat"].T).astype(BF16_NP)
    vfT = np.ascontiguousarray(args["v_feat"].T).astype(BF16_NP)
    wq8 = (args["Wq"] / SCALE).astype(BF16_NP)
    wkb = args["Wk"].astype(BF16_NP)
    wvb = args["Wv"].astype(BF16_NP)
    wob = np.ascontiguousarray(args["Wo"]).astype(BF16_NP)

    in_maps = []
    for c in range(NCORES):
        qs, qe = int(qbound[c]), int(qbound[c + 1])
        ks, ke = int(kbound[c]), int(kbound[c + 1])
        nq, nk = qe - qs, ke - ks

        qfc = np.zeros((QD, NQ), BF16_NP)
        qfc[:, :nq] = qfT[:, qs:qe]
        kfc = np.zeros((QD, NK), BF16_NP)
        kfc[:, :nk] = kfT[:, ks:ke]
        vfc = np.zeros((QD, NK), BF16_NP)
        vfc[:, :nk] = vfT[:, ks:ke]

        posc = np.full((H, NK, NQ), -1e9, BF16_NP)
        if nk > 0:
            posc[:, :nk, :] = 0.0   # real k rows: unmasked vs padded q cols
            posc[:, :nk, :nq] = args["pos_enc"][:, qs:qe, ks:ke] \
                .swapaxes(1, 2).astype(BF16_NP)

        m = {"qfT": qfc, "kfT": kfc, "vfT": vfc, "posc": posc,
             "wq": wq8, "wk": wkb, "wv": wvb, "wo": wob}
        if has_bq:
            m["bq"] = (args["bq"] / SCALE).astype(BF16_NP).reshape(1, OD)
        if has_bk:
            m["bk"] = args["bk"].astype(BF16_NP).reshape(1, OD)
        if has_bv:
            m["bv"] = args["bv"].astype(BF16_NP).reshape(1, OD)
        if has_bo:
            m["bo"] = np.ascontiguousarray(
                args["bo"].astype(np.float32).reshape(OD // 128, 128).T)
        in_maps.append(m)

    res = run_bass_kernel_spmd(nc, in_maps, core_ids=list(range(NCORES)),
                               trace=TRACE)
    LAST_RESULTS = res
    out = np.empty((N, OD), np.float32)
    for c in range(NCORES):
        qs, qe = int(qbound[c]), int(qbound[c + 1])
        if qe > qs:
            out[qs:qe, :] = res.results[c]["out"][:, :qe - qs].T
    return out


# revision 37
# speedup vs baseline: 2.4791x; 1.0314x over previous
"""Trainium2 Bass kernel for nn_MultiHeadAttention_3796751090171 (sparse_attention).

Batch-parallel SPMD across 8 NeuronCores: q_batch/k_batch are SORTED, so the
cross-batch mask makes attention block-diagonal over batches, and there are
exactly B=8 batches for 8 cores. Core c computes batch c's queries against
batch c's keys for ALL 8 heads -- completely independent work, so there are
NO collectives: the full output is a pure row-concatenation of the per-core
outputs.

Uniform SPMD template: every core runs the same program on [NKMAX x NQMAX]
padded tiles (NQMAX/NKMAX = max batch size rounded up to 128). The host pads
each core's feature slices with zeros and fabricates the pos_enc tile so that
  - padded k-rows carry pos = -1e9  -> exp = 0, no contribution to h or Z
  - padded q-cols carry pos = 0 on real k-rows (keeps Z finite; outputs for
    those columns are dropped by the host)

Per core c (batch slice qs:qe / ks:ke, all heads h):
  Q = qf[qs:qe] @ Wq/8, K = kf[ks:ke] @ Wk, V = vf[ks:ke] @ Wv  (+biases)
  per head: scoresT[k,q] = K_h^T-chunks @ Q_h + posT  (PSUM)
  expT = exp(scoresT); hT_unnorm/Z via [V|1] matmul (ones column -> row 64 = Z)
  hTn = hT * (1/Z broadcast); out^T[o,q] = sum_h Wo[64h:,:].T @ hTn_h + bo
Host: out[qs:qe, :] = outT[:, :nq].T

No max-subtraction in softmax: scores are O(10) so exp is safe in fp32; masked
entries give exp(-1e9+...) -> exactly 0, matching the reference's
exp(-1e9 - max) -> 0.
"""

import functools
import math

import numpy as np
import ml_dtypes

import concourse.bass as bass
import concourse.tile as tile
from concourse import bacc, mybir
from concourse.bass_utils import run_bass_kernel_spmd
from concourse.masks import make_identity

N = 3072
QD = 512
OD = 512
H = 8
D = 64
B = 8
NCORES = 8
SCALE = math.sqrt(D)

F32 = mybir.dt.float32
BF16 = mybir.dt.bfloat16
BF16_NP = ml_dtypes.bfloat16

TRACE = False
LAST_RESULTS = None


def _bounds(q_batch, k_batch):
    qb = np.asarray(q_batch).astype(np.int64)
    kb = np.asarray(k_batch).astype(np.int64)
    qbound = np.searchsorted(qb, np.arange(B + 1))
    kbound = np.searchsorted(kb, np.arange(B + 1))
    return qbound, kbound


def _chunks(lo, hi, step):
    return [(o, min(step, hi - o)) for o in range(lo, hi, step)]


def _r128(x):
    return max(128, ((x + 127) // 128) * 128)


@functools.lru_cache(maxsize=8)
def _build(NQ, NK, has_bq, has_bk, has_bv, has_bo):
    nc = bacc.Bacc("TRN2", target_bir_lowering=False, debug=False,
                   num_devices=NCORES)

    KT_T = QD // 128   # 4 contraction tiles for the projections
    NKC = NK // 128    # k chunks
    QCH = _chunks(0, NQ, 512)   # q chunks (free-dim <= 512)
    NTD = QD // 128    # output-d tiles for projections

    qfT_d = nc.dram_tensor("qfT", [QD, NQ], BF16, kind="ExternalInput")
    kfT_d = nc.dram_tensor("kfT", [QD, NK], BF16, kind="ExternalInput")
    vfT_d = nc.dram_tensor("vfT", [QD, NK], BF16, kind="ExternalInput")
    posc_d = nc.dram_tensor("posc", [H, NK, NQ], BF16, kind="ExternalInput")
    wq_d = nc.dram_tensor("wq", [QD, OD], BF16, kind="ExternalInput")
    wk_d = nc.dram_tensor("wk", [QD, OD], BF16, kind="ExternalInput")
    wv_d = nc.dram_tensor("wv", [QD, OD], BF16, kind="ExternalInput")
    wo_d = nc.dram_tensor("wo", [OD, OD], BF16, kind="ExternalInput")
    bq_d = nc.dram_tensor("bq", [1, OD], BF16, kind="ExternalInput") if has_bq else None
    bk_d = nc.dram_tensor("bk", [1, OD], BF16, kind="ExternalInput") if has_bk else None
    bv_d = nc.dram_tensor("bv", [1, OD], BF16, kind="ExternalInput") if has_bv else None
    bo_d = nc.dram_tensor("bo", [128, NTD], F32, kind="ExternalInput") if has_bo else None
    out_d = nc.dram_tensor("out", [OD, NQ], F32, kind="ExternalOutput")
    import os
    DEBUG = bool(os.environ.get("KDBG"))
    if DEBUG:
        dbg_h = nc.dram_tensor("dbg_h", [D, H, NQ], F32, kind="ExternalOutput")
        dbg_z = nc.dram_tensor("dbg_z", [1, H, NQ], F32, kind="ExternalOutput")
        dbg_e = nc.dram_tensor("dbg_e", [128, NQ], F32, kind="ExternalOutput")

    with tile.TileContext(nc) as tc:
        with (
            tc.tile_pool(name="consts", bufs=1) as consts,
            tc.tile_pool(name="pos", bufs=12) as posp,
            tc.tile_pool(name="expp", bufs=12) as expp,
            tc.tile_pool(name="outp", bufs=4) as outp,
            tc.tile_pool(name="ps_s", bufs=4, space="PSUM") as ps_s,
            tc.tile_pool(name="ps_h", bufs=2, space="PSUM") as ps_h,
            tc.tile_pool(name="ps_p", bufs=2, space="PSUM") as ps_p,
            tc.tile_pool(name="dram", bufs=1, space="DRAM") as dramp,
        ):
            # ---------------- constants / weights ----------------
            ones = consts.tile([1, max(NQ, NK)], BF16)
            nc.vector.memset(ones, 1.0)
            ones_f = consts.tile([1, D], F32)
            nc.vector.memset(ones_f, 1.0)
            ident64 = consts.tile([D, D], BF16)
            make_identity(nc, ident64)

            wq_sb = consts.tile([128, KT_T, OD], BF16)
            wk_sb = consts.tile([128, KT_T, OD], BF16)
            wv_sb = consts.tile([128, KT_T, OD], BF16)
            for t_d, t_sb in ((wq_d, wq_sb), (wk_d, wk_sb), (wv_d, wv_sb)):
                nc.sync.dma_start(
                    out=t_sb, in_=t_d.ap().rearrange("(t p) d -> p t d", p=128))
            # wo as [64, h, oc, 128] for contraction-64 output projection
            wo_sb = consts.tile([D, H, NTD, 128], BF16)
            nc.gpsimd.dma_start(
                out=wo_sb,
                in_=wo_d.ap().rearrange("(h p) (o c) -> p h o c", p=D, c=128))
            bias_sb = {}
            for nm, dd in (("bq", bq_d), ("bk", bk_d), ("bv", bv_d)):
                if dd is not None:
                    t = consts.tile([1, OD], BF16, tag=f"bias_{nm}", name=f"b_{nm}")
                    nc.gpsimd.dma_start(out=t, in_=dd[:, :])
                    bias_sb[nm] = t
            if bo_d is not None:
                bo_sb = consts.tile([128, NTD], F32)
                nc.gpsimd.dma_start(out=bo_sb, in_=bo_d[:, :])

            # feature tiles
            qf_sb = consts.tile([128, KT_T, NQ], BF16)
            kf_sb = consts.tile([128, KT_T, NK], BF16)
            vf_sb = consts.tile([128, KT_T, NK], BF16)
            for f_d, f_sb in ((qfT_d, qf_sb), (kfT_d, kf_sb), (vfT_d, vf_sb)):
                for t in range(KT_T):
                    nc.sync.dma_start(
                        out=f_sb[:, t, :],
                        in_=f_d.ap().rearrange("(t p) n -> t p n", p=128)[t])

            # projected tensors, split by head parity so every matmul operand
            # sits at partition base 0 (slot index = head // 2)
            QT_e = consts.tile([D, NTD, NQ], BF16, name="QT_e")
            QT_o = consts.tile([D, NTD, NQ], BF16, name="QT_o")
            KT_e = consts.tile([D, NTD, NK], BF16, name="KT_e")
            KT_o = consts.tile([D, NTD, NK], BF16, name="KT_o")
            VT_e = consts.tile([D, NTD, NK], BF16, name="VT_e")
            VT_o = consts.tile([D, NTD, NK], BF16, name="VT_o")
            V_sb = consts.tile([128, H, NKC, D + 1], BF16, name="V_sb")
            hT_sb = consts.tile([D, H, NQ], BF16, name="hT_sb")
            hTn_sb = consts.tile([D, H, NQ], BF16, name="hTn_sb")
            zall_sb = consts.tile([1, H, NQ], F32, name="zall_sb")
            zrec_sb = consts.tile([1, H, NQ], F32, name="zrec_sb")

            # ---------------- projections ----------------
            def project(f_sb, w_sb, bias, dstE, dstO, xchunks):
                for td in range(NTD):
                    dsl = slice(128 * td, 128 * (td + 1))
                    for (xo, xw) in xchunks:
                        xsl = slice(xo, xo + xw)
                        psum = ps_p.tile([128, 512], F32, tag="psp")
                        for t in range(KT_T):
                            nc.tensor.matmul(psum[:, 0:xw],
                                             w_sb[:, t, dsl], f_sb[:, t, xsl],
                                             start=(t == 0),
                                             stop=(t == KT_T - 1 and bias is None))
                        if bias is not None:
                            nc.tensor.matmul(psum[:, 0:xw], bias[:, dsl],
                                             ones[:, xsl], start=False, stop=True)
                        nc.vector.tensor_copy(dstE[:, td, xsl], psum[0:D, 0:xw])
                        nc.scalar.copy(dstO[:, td, xsl], psum[D:128, 0:xw])

            kchunks = _chunks(0, NK, 512)
            project(qf_sb, wq_sb, bias_sb.get("bq"), QT_e, QT_o, QCH)
            project(kf_sb, wk_sb, bias_sb.get("bk"), KT_e, KT_o, kchunks)
            project(vf_sb, wv_sb, bias_sb.get("bv"), VT_e, VT_o, kchunks)

            # V into [k, d | ones] per (head, kchunk) via PE transposes
            nc.vector.memset(V_sb[:, :, :, D], 1.0)
            for h in range(H):
                VT = VT_e if h % 2 == 0 else VT_o
                for kc in range(NKC):
                    ksl = slice(128 * kc, 128 * (kc + 1))
                    pst = ps_p.tile([128, 512], BF16, tag="psp")
                    nc.tensor.transpose(pst[:, 0:D], VT[:, h // 2, ksl],
                                        ident64[:, :])
                    nc.scalar.copy(V_sb[:, h, kc, 0:D], pst[:, 0:D])

            # ---------------- attention, software-pipelined ----------------
            units = [(h, qo, qw) for h in range(H) for (qo, qw) in QCH]
            expts = {}

            def stage1(i):
                h, qo, qw = units[i]
                qsl = slice(qo, qo + qw)
                QT = QT_e if h % 2 == 0 else QT_o
                KT = KT_e if h % 2 == 0 else KT_o
                lst = []
                for kc in range(NKC):
                    ksl = slice(128 * kc, 128 * (kc + 1))
                    ps = ps_s.tile([128, 512], F32, tag="pss")
                    nc.tensor.matmul(ps[:, 0:qw], KT[:, h // 2, ksl],
                                     QT[:, h // 2, qsl], start=True, stop=True)
                    pos = posp.tile([128, 512], BF16, tag="pos")
                    nc.gpsimd.dma_start(out=pos[:, 0:qw], in_=posc_d[h, ksl, qsl])
                    nc.vector.tensor_add(ps[:, 0:qw], ps[:, 0:qw], pos[:, 0:qw])
                    expt = expp.tile([128, 512], BF16, tag="expt")
                    nc.scalar.activation(expt[:, 0:qw], ps[:, 0:qw],
                                         mybir.ActivationFunctionType.Exp)
                    if DEBUG and i == 0 and kc == 0:
                        de = consts.tile([128, NQ], F32, name="de")
                        nc.vector.tensor_copy(de[:, 0:qw], expt[:, 0:qw])
                        nc.gpsimd.dma_start(out=dbg_e.ap(), in_=de[:, :])
                    lst.append(expt)
                expts[i] = lst

            def stage2(i):
                h, qo, qw = units[i]
                qsl = slice(qo, qo + qw)
                psum_h = ps_h.tile([D + 1, 512], F32, tag="psh")
                for kc in range(NKC):
                    nc.tensor.matmul(psum_h[:, 0:qw], V_sb[:, h, kc, :],
                                     expts[i][kc][:, 0:qw],
                                     start=(kc == 0), stop=(kc == NKC - 1))
                del expts[i]
                nc.scalar.copy(hT_sb[:, h, qsl], psum_h[0:D, 0:qw])
                # 1/Z for this unit (Z = row 64 of the accumulator); the
                # approx reciprocal requires an SBUF source, so copy first
                nc.scalar.copy(zall_sb[0:1, h, qsl], psum_h[D:D + 1, 0:qw])
                nc.vector.reciprocal_approx_fast(zrec_sb[0:1, h, qsl],
                                                 zall_sb[0:1, h, qsl])
                # broadcast 1/Z across the 64 d-partitions with a K=1 matmul
                # and normalize this unit's hT in place
                zb_ps = ps_p.tile([128, 512], F32, tag="psp")
                nc.tensor.matmul(zb_ps[0:D, 0:qw], ones_f[0:1, :],
                                 zrec_sb[0:1, h, qsl], start=True, stop=True)
                nc.vector.tensor_mul(hTn_sb[:, h, qsl], hT_sb[:, h, qsl],
                                     zb_ps[0:D, 0:qw])

            LOOK = 2
            for i in range(len(units)):
                stage1(i)
                if i >= LOOK:
                    stage2(i - LOOK)
            for i in range(max(0, len(units) - LOOK), len(units)):
                stage2(i)

            if DEBUG:
                dh = consts.tile([D, H, NQ], F32, name="dh")
                nc.vector.tensor_copy(dh[:, :, :], hT_sb[:, :, :])
                nc.gpsimd.dma_start(out=dbg_h.ap(), in_=dh[:, :, :])
                nc.gpsimd.dma_start(out=dbg_z.ap(), in_=zrec_sb[0:1, :, :])

            # ---------------- output projection ----------------
            for oc in range(NTD):
                for (qo, qw) in QCH:
                    qsl = slice(qo, qo + qw)
                    psum = ps_p.tile([128, 512], F32, tag="psp")
                    for h in range(H):
                        nc.tensor.matmul(psum[:, 0:qw], wo_sb[:, h, oc, :],
                                         hTn_sb[:, h, qsl],
                                         start=(h == 0), stop=(h == H - 1))
                    o_sb = outp.tile([128, 512], F32, tag="osb")
                    if bo_d is not None:
                        nc.scalar.activation(o_sb[:, 0:qw], psum[:, 0:qw],
                                             mybir.ActivationFunctionType.Identity,
                                             bias=bo_sb[:, oc:oc + 1])
                    else:
                        nc.vector.tensor_copy(o_sb[:, 0:qw], psum[:, 0:qw])
                    nc.gpsimd.dma_start(out=out_d[128 * oc:128 * (oc + 1), qsl],
                                        in_=o_sb[:, 0:qw])

    nc.compile()
    return nc


def _kernel_numpy(q_feat, k_feat, v_feat, pos_enc, Wq, bq, Wk, bk, Wv, bv,
                  Wo, bo, q_batch, k_batch):
    """Host fallback (degenerate batch layouts) + debugging aid."""
    Q = (q_feat @ Wq + bq).reshape(N, H, D).transpose(1, 0, 2)
    K = (k_feat @ Wk + bk).reshape(N, H, D).transpose(1, 0, 2)
    V = (v_feat @ Wv + bv).reshape(N, H, D).transpose(1, 0, 2)
    scores = np.einsum("hnd,hmd->hnm", Q, K) / SCALE + pos_enc
    mask = q_batch[:, None] != k_batch[None, :]
    scores = np.where(mask[None], np.float32(-1e9), scores)
    scores = scores - scores.max(-1, keepdims=True)
    e = np.exp(scores)
    probs = e / e.sum(-1, keepdims=True)
    h = np.einsum("hnm,hmd->hnd", probs, V)
    h = h.transpose(1, 0, 2).reshape(N, OD)
    return (h @ Wo + bo).astype(np.float32)


def kernel(q_feat, k_feat, v_feat, pos_enc, Wq, bq, Wk, bk, Wv, bv, Wo, bo,
           q_batch, k_batch):
    global LAST_RESULTS
    args = dict(q_feat=np.asarray(q_feat, np.float32),
                k_feat=np.asarray(k_feat, np.float32),
                v_feat=np.asarray(v_feat, np.float32),
                pos_enc=np.asarray(pos_enc, np.float32),
                Wq=np.asarray(Wq, np.float32), bq=np.asarray(bq, np.float32),
                Wk=np.asarray(Wk, np.float32), bk=np.asarray(bk, np.float32),
                Wv=np.asarray(Wv, np.float32), bv=np.asarray(bv, np.float32),
                Wo=np.asarray(Wo, np.float32), bo=np.asarray(bo, np.float32),
                q_batch=np.asarray(q_batch), k_batch=np.asarray(k_batch))

    qbound, kbound = _bounds(args["q_batch"], args["k_batch"])
    nq_all = np.diff(qbound)
    nk_all = np.diff(kbound)
    if np.any((nq_all > 0) & (nk_all == 0)):
        # some batch has queries but no keys: reference gives uniform attention
        # over ALL keys there; fall back (never happens with realistic inputs)
        return _kernel_numpy(**args)

    NQ = _r128(int(nq_all.max()))
    NK = _r128(int(nk_all.max()))

    has_bq = bool(np.any(args["bq"]))
    has_bk = bool(np.any(args["bk"]))
    has_bv = bool(np.any(args["bv"]))
    has_bo = bool(np.any(args["bo"]))

    nc = _build(NQ, NK, has_bq, has_bk, has_bv, has_bo)

    # ---- host-side sharding / layout / padding ----
    qfT = np.ascontiguousarray(args["q_feat"].T).astype(BF16_NP)
    kfT = np.ascontiguousarray(args["k_feat"].T).astype(BF16_NP)
    vfT = np.ascontiguousarray(args["v_feat"].T).astype(BF16_NP)
    wq8 = (args["Wq"] / SCALE).astype(BF16_NP)
    wkb = args["Wk"].astype(BF16_NP)
    wvb = args["Wv"].astype(BF16_NP)
    wob = np.ascontiguousarray(args["Wo"]).astype(BF16_NP)

    in_maps = []
    for c in range(NCORES):
        qs, qe = int(qbound[c]), int(qbound[c + 1])
        ks, ke = int(kbound[c]), int(kbound[c + 1])
        nq, nk = qe - qs, ke - ks

        qfc = np.zeros((QD, NQ), BF16_NP)
        qfc[:, :nq] = qfT[:, qs:qe]
        kfc = np.zeros((QD, NK), BF16_NP)
        kfc[:, :nk] = kfT[:, ks:ke]
        vfc = np.zeros((QD, NK), BF16_NP)
        vfc[:, :nk] = vfT[:, ks:ke]

        posc = np.full((H, NK, NQ), -1e9, BF16_NP)
        if nk > 0:
            posc[:, :nk, :] = 0.0   # real k rows: unmasked vs padded q cols
            posc[:, :nk, :nq] = args["pos_enc"][:, qs:qe, ks:ke] \
                .swapaxes(1, 2).astype(BF16_NP)

        m = {"qfT": qfc, "kfT": kfc, "vfT": vfc, "posc": posc,
             "wq": wq8, "wk": wkb, "wv": wvb, "wo": wob}
        if has_bq:
            m["bq"] = (args["bq"] / SCALE).astype(BF16_NP).reshape(1, OD)
        if has_bk:
            m["bk"] = args["bk"].astype(BF16_NP).reshape(1, OD)
        if has_bv:
            m["bv"] = args["bv"].astype(BF16_NP).reshape(1, OD)
        if has_bo:
            m["bo"] = np.ascontiguousarray(
                args["bo"].astype(np.float32).reshape(OD // 128, 128).T)
        in_maps.append(m)

    res = run_bass_kernel_spmd(nc, in_maps, core_ids=list(range(NCORES)),
                               trace=TRACE)
    LAST_RESULTS = res
    out = np.empty((N, OD), np.float32)
    for c in range(NCORES):
        qs, qe = int(qbound[c]), int(qbound[c + 1])
        if qe > qs:
            out[qs:qe, :] = res.results[c]["out"][:, :qe - qs].T
    return out


# revision 38
# speedup vs baseline: 2.8533x; 1.1509x over previous
"""Trainium2 Bass kernel for nn_MultiHeadAttention_3796751090171 (sparse_attention).

Batch-parallel SPMD across 8 NeuronCores: q_batch/k_batch are SORTED, so the
cross-batch mask makes attention block-diagonal over batches, and there are
exactly B=8 batches for 8 cores. Core c computes batch c's queries against
batch c's keys for ALL 8 heads -- completely independent work, so there are
NO collectives: the full output is a pure row-concatenation of the per-core
outputs.

Uniform SPMD template: every core runs the same program on [NKMAX x NQMAX]
padded tiles (NQMAX/NKMAX = max batch size rounded up to 128). The host pads
each core's feature slices with zeros and fabricates the pos_enc tile so that
  - padded k-rows carry pos = -1e9  -> exp = 0, no contribution to h or Z
  - padded q-cols carry pos = 0 on real k-rows (keeps Z finite; outputs for
    those columns are dropped by the host)

Per core c (batch slice qs:qe / ks:ke, all heads h):
  Q = qf[qs:qe] @ Wq/8, K = kf[ks:ke] @ Wk, V = vf[ks:ke] @ Wv  (+biases)
  per head: scoresT[k,q] = K_h^T-chunks @ Q_h + posT  (PSUM)
  expT = exp(scoresT); hT_unnorm/Z via [V|1] matmul (ones column -> row 64 = Z)
  hTn = hT * (1/Z broadcast); out^T[o,q] = sum_h Wo[64h:,:].T @ hTn_h + bo
Host: out[qs:qe, :] = outT[:, :nq].T

No max-subtraction in softmax: scores are O(10) so exp is safe in fp32; masked
entries give exp(-1e9+...) -> exactly 0, matching the reference's
exp(-1e9 - max) -> 0.
"""

import functools
import math

import numpy as np
import ml_dtypes

import concourse.bass as bass
import concourse.tile as tile
from concourse import bacc, mybir
from concourse.bass_utils import run_bass_kernel_spmd
from concourse.masks import make_identity

N = 3072
QD = 512
OD = 512
H = 8
D = 64
B = 8
NCORES = 8
SCALE = math.sqrt(D)

F32 = mybir.dt.float32
BF16 = mybir.dt.bfloat16
BF16_NP = ml_dtypes.bfloat16

TRACE = False
LAST_RESULTS = None


def _bounds(q_batch, k_batch):
    qb = np.asarray(q_batch).astype(np.int64)
    kb = np.asarray(k_batch).astype(np.int64)
    qbound = np.searchsorted(qb, np.arange(B + 1))
    kbound = np.searchsorted(kb, np.arange(B + 1))
    return qbound, kbound


def _chunks(lo, hi, step):
    return [(o, min(step, hi - o)) for o in range(lo, hi, step)]


def _r128(x):
    return max(128, ((x + 127) // 128) * 128)


@functools.lru_cache(maxsize=8)
def _build(NQ, NK, has_bq, has_bk, has_bv, has_bo):
    nc = bacc.Bacc("TRN2", target_bir_lowering=False, debug=False,
                   num_devices=NCORES)

    KT_T = QD // 128   # 4 contraction tiles for the projections
    NKC = NK // 128    # k chunks
    QCH = _chunks(0, NQ, 512)   # q chunks (free-dim <= 512)
    NTD = QD // 128    # output-d tiles for projections

    qfT_d = nc.dram_tensor("qfT", [QD, NQ], BF16, kind="ExternalInput")
    kfT_d = nc.dram_tensor("kfT", [QD, NK], BF16, kind="ExternalInput")
    vfT_d = nc.dram_tensor("vfT", [QD, NK], BF16, kind="ExternalInput")
    posc_d = nc.dram_tensor("posc", [H, NK, NQ], BF16, kind="ExternalInput")
    wq_d = nc.dram_tensor("wq", [QD, OD], BF16, kind="ExternalInput")
    wk_d = nc.dram_tensor("wk", [QD, OD], BF16, kind="ExternalInput")
    wv_d = nc.dram_tensor("wv", [QD, OD], BF16, kind="ExternalInput")
    wo_d = nc.dram_tensor("wo", [OD, OD], BF16, kind="ExternalInput")
    bq_d = nc.dram_tensor("bq", [1, OD], BF16, kind="ExternalInput") if has_bq else None
    bk_d = nc.dram_tensor("bk", [1, OD], BF16, kind="ExternalInput") if has_bk else None
    bv_d = nc.dram_tensor("bv", [1, OD], BF16, kind="ExternalInput") if has_bv else None
    bo_d = nc.dram_tensor("bo", [128, NTD], F32, kind="ExternalInput") if has_bo else None
    out_d = nc.dram_tensor("out", [OD, NQ], F32, kind="ExternalOutput")
    import os
    DEBUG = bool(os.environ.get("KDBG"))
    if DEBUG:
        dbg_h = nc.dram_tensor("dbg_h", [D, H, NQ], F32, kind="ExternalOutput")
        dbg_z = nc.dram_tensor("dbg_z", [1, H, NQ], F32, kind="ExternalOutput")
        dbg_e = nc.dram_tensor("dbg_e", [128, NQ], F32, kind="ExternalOutput")

    with tile.TileContext(nc) as tc:
        with (
            tc.tile_pool(name="consts", bufs=1) as consts,
            tc.tile_pool(name="pos", bufs=12) as posp,
            tc.tile_pool(name="expp", bufs=12) as expp,
            tc.tile_pool(name="outp", bufs=4) as outp,
            tc.tile_pool(name="ps_s", bufs=4, space="PSUM") as ps_s,
            tc.tile_pool(name="ps_h", bufs=2, space="PSUM") as ps_h,
            tc.tile_pool(name="ps_p", bufs=2, space="PSUM") as ps_p,
            tc.tile_pool(name="dram", bufs=1, space="DRAM") as dramp,
        ):
            # ---------------- constants / weights ----------------
            ones = consts.tile([1, max(NQ, NK)], BF16)
            nc.vector.memset(ones, 1.0)
            ones_f = consts.tile([1, D], F32)
            nc.vector.memset(ones_f, 1.0)
            ident128 = consts.tile([128, 128], BF16)
            make_identity(nc, ident128)

            wq_sb = consts.tile([128, KT_T, OD], BF16)
            wk_sb = consts.tile([128, KT_T, OD], BF16)
            wv_sb = consts.tile([128, KT_T, OD], BF16)
            for t_d, t_sb in ((wq_d, wq_sb), (wk_d, wk_sb), (wv_d, wv_sb)):
                nc.sync.dma_start(
                    out=t_sb, in_=t_d.ap().rearrange("(t p) d -> p t d", p=128))
            # wo as [128, t, oc, 128] for contraction-128 output projection
            wo_sb = consts.tile([128, NTD, NTD, 128], BF16)
            nc.gpsimd.dma_start(
                out=wo_sb,
                in_=wo_d.ap().rearrange("(t p) (o c) -> p t o c", p=128, c=128))
            bias_sb = {}
            for nm, dd in (("bq", bq_d), ("bk", bk_d), ("bv", bv_d)):
                if dd is not None:
                    t = consts.tile([1, OD], BF16, tag=f"bias_{nm}", name=f"b_{nm}")
                    nc.gpsimd.dma_start(out=t, in_=dd[:, :])
                    bias_sb[nm] = t
            if bo_d is not None:
                bo_sb = consts.tile([128, NTD], F32)
                nc.gpsimd.dma_start(out=bo_sb, in_=bo_d[:, :])

            # feature tiles
            qf_sb = consts.tile([128, KT_T, NQ], BF16)
            kf_sb = consts.tile([128, KT_T, NK], BF16)
            vf_sb = consts.tile([128, KT_T, NK], BF16)
            for f_d, f_sb in ((qfT_d, qf_sb), (kfT_d, kf_sb), (vfT_d, vf_sb)):
                for t in range(KT_T):
                    nc.sync.dma_start(
                        out=f_sb[:, t, :],
                        in_=f_d.ap().rearrange("(t p) n -> t p n", p=128)[t])

            # projected tensors, split by head parity so every matmul operand
            # sits at partition base 0 (slot index = head // 2)
            QT_e = consts.tile([D, NTD, NQ], BF16, name="QT_e")
            QT_o = consts.tile([D, NTD, NQ], BF16, name="QT_o")
            KT_e = consts.tile([D, NTD, NK], BF16, name="KT_e")
            KT_o = consts.tile([D, NTD, NK], BF16, name="KT_o")
            VT_f = consts.tile([128, NTD, NK], BF16, name="VT_f")
            V_sb = consts.tile([128, NKC, H, D + 1], BF16, name="V_sb")
            hT_sb = consts.tile([D, H, NQ], BF16, name="hT_sb")
            hTn_sb = consts.tile([128, NTD, NQ], BF16, name="hTn_sb")
            zall_sb = consts.tile([1, H, NQ], F32, name="zall_sb")
            zrec_sb = consts.tile([1, H, NQ], F32, name="zrec_sb")

            # ---------------- projections ----------------
            def project(f_sb, w_sb, bias, dstE, dstO, xchunks):
                for td in range(NTD):
                    dsl = slice(128 * td, 128 * (td + 1))
                    for (xo, xw) in xchunks:
                        xsl = slice(xo, xo + xw)
                        psum = ps_p.tile([128, 512], F32, tag="psp")
                        for t in range(KT_T):
                            nc.tensor.matmul(psum[:, 0:xw],
                                             w_sb[:, t, dsl], f_sb[:, t, xsl],
                                             start=(t == 0),
                                             stop=(t == KT_T - 1 and bias is None))
                        if bias is not None:
                            nc.tensor.matmul(psum[:, 0:xw], bias[:, dsl],
                                             ones[:, xsl], start=False, stop=True)
                        nc.vector.tensor_copy(dstE[:, td, xsl], psum[0:D, 0:xw])
                        nc.scalar.copy(dstO[:, td, xsl], psum[D:128, 0:xw])

            kchunks = _chunks(0, NK, 512)
            project(qf_sb, wq_sb, bias_sb.get("bq"), QT_e, QT_o, QCH)
            project(kf_sb, wk_sb, bias_sb.get("bk"), KT_e, KT_o, kchunks)

            def project_full(f_sb, w_sb, bias, dst, xchunks):
                for td in range(NTD):
                    dsl = slice(128 * td, 128 * (td + 1))
                    for (xo, xw) in xchunks:
                        xsl = slice(xo, xo + xw)
                        psum = ps_p.tile([128, 512], F32, tag="psp")
                        for t in range(KT_T):
                            nc.tensor.matmul(psum[:, 0:xw],
                                             w_sb[:, t, dsl], f_sb[:, t, xsl],
                                             start=(t == 0),
                                             stop=(t == KT_T - 1 and bias is None))
                        if bias is not None:
                            nc.tensor.matmul(psum[:, 0:xw], bias[:, dsl],
                                             ones[:, xsl], start=False, stop=True)
                        nc.scalar.copy(dst[:, td, xsl], psum[:, 0:xw])

            project_full(vf_sb, wv_sb, bias_sb.get("bv"), VT_f, kchunks)

            # V into [k, d | ones] per (kchunk, d-tile) via full PE transposes;
            # each [128,128] transpose covers two heads' 64-dim halves
            nc.vector.memset(V_sb[:, :, :, D], 1.0)
            for kc in range(NKC):
                ksl = slice(128 * kc, 128 * (kc + 1))
                for td in range(NTD):
                    pst = ps_p.tile([128, 512], BF16, tag="psp")
                    nc.tensor.transpose(pst[:, 0:128], VT_f[:, td, ksl],
                                        ident128[:, :])
                    nc.scalar.copy(V_sb[:, kc, 2 * td, 0:D], pst[:, 0:D])
                    nc.vector.tensor_copy(V_sb[:, kc, 2 * td + 1, 0:D],
                                          pst[:, D:128])

            # ---------------- attention, software-pipelined ----------------
            units = [(h, qo, qw) for h in range(H) for (qo, qw) in QCH]
            expts = {}

            def stage1(i):
                h, qo, qw = units[i]
                qsl = slice(qo, qo + qw)
                QT = QT_e if h % 2 == 0 else QT_o
                KT = KT_e if h % 2 == 0 else KT_o
                lst = []
                for kc in range(NKC):
                    ksl = slice(128 * kc, 128 * (kc + 1))
                    ps = ps_s.tile([128, 512], F32, tag="pss")
                    nc.tensor.matmul(ps[:, 0:qw], KT[:, h // 2, ksl],
                                     QT[:, h // 2, qsl], start=True, stop=True)
                    pos = posp.tile([128, 512], BF16, tag="pos")
                    nc.gpsimd.dma_start(out=pos[:, 0:qw], in_=posc_d[h, ksl, qsl])
                    nc.vector.tensor_add(ps[:, 0:qw], ps[:, 0:qw], pos[:, 0:qw])
                    expt = expp.tile([128, 512], BF16, tag="expt")
                    nc.scalar.activation(expt[:, 0:qw], ps[:, 0:qw],
                                         mybir.ActivationFunctionType.Exp)
                    if DEBUG and i == 0 and kc == 0:
                        de = consts.tile([128, NQ], F32, name="de")
                        nc.vector.tensor_copy(de[:, 0:qw], expt[:, 0:qw])
                        nc.gpsimd.dma_start(out=dbg_e.ap(), in_=de[:, :])
                    lst.append(expt)
                expts[i] = lst

            def stage2(i):
                h, qo, qw = units[i]
                qsl = slice(qo, qo + qw)
                psum_h = ps_h.tile([D + 1, 512], F32, tag="psh")
                for kc in range(NKC):
                    nc.tensor.matmul(psum_h[:, 0:qw], V_sb[:, kc, h, :],
                                     expts[i][kc][:, 0:qw],
                                     start=(kc == 0), stop=(kc == NKC - 1))
                del expts[i]
                nc.scalar.copy(hT_sb[:, h, qsl], psum_h[0:D, 0:qw])
                # 1/Z for this unit (Z = row 64 of the accumulator); the
                # approx reciprocal requires an SBUF source, so copy first
                nc.scalar.copy(zall_sb[0:1, h, qsl], psum_h[D:D + 1, 0:qw])
                nc.vector.reciprocal_approx_fast(zrec_sb[0:1, h, qsl],
                                                 zall_sb[0:1, h, qsl])
                # broadcast 1/Z across the 64 d-partitions with a K=1 matmul
                # and normalize this unit's hT in place
                zb_ps = ps_p.tile([128, 512], F32, tag="psp")
                nc.tensor.matmul(zb_ps[0:D, 0:qw], ones_f[0:1, :],
                                 zrec_sb[0:1, h, qsl], start=True, stop=True)
                po = D * (h % 2)
                nc.vector.tensor_mul(hTn_sb[po:po + D, h // 2, qsl],
                                     hT_sb[:, h, qsl], zb_ps[0:D, 0:qw])

            LOOK = 2
            for i in range(len(units)):
                stage1(i)
                if i >= LOOK:
                    stage2(i - LOOK)
            for i in range(max(0, len(units) - LOOK), len(units)):
                stage2(i)

            if DEBUG:
                dh = consts.tile([D, H, NQ], F32, name="dh")
                nc.vector.tensor_copy(dh[:, :, :], hT_sb[:, :, :])
                nc.gpsimd.dma_start(out=dbg_h.ap(), in_=dh[:, :, :])
                nc.gpsimd.dma_start(out=dbg_z.ap(), in_=zrec_sb[0:1, :, :])

            # ---------------- output projection ----------------
            for oc in range(NTD):
                for (qo, qw) in QCH:
                    qsl = slice(qo, qo + qw)
                    psum = ps_p.tile([128, 512], F32, tag="psp")
                    for t in range(NTD):
                        nc.tensor.matmul(psum[:, 0:qw], wo_sb[:, t, oc, :],
                                         hTn_sb[:, t, qsl],
                                         start=(t == 0), stop=(t == NTD - 1))
                    o_sb = outp.tile([128, 512], F32, tag="osb")
                    if bo_d is not None:
                        nc.scalar.activation(o_sb[:, 0:qw], psum[:, 0:qw],
                                             mybir.ActivationFunctionType.Identity,
                                             bias=bo_sb[:, oc:oc + 1])
                    else:
                        nc.vector.tensor_copy(o_sb[:, 0:qw], psum[:, 0:qw])
                    nc.gpsimd.dma_start(out=out_d[128 * oc:128 * (oc + 1), qsl],
                                        in_=o_sb[:, 0:qw])

    nc.compile()
    return nc


def _kernel_numpy(q_feat, k_feat, v_feat, pos_enc, Wq, bq, Wk, bk, Wv, bv,
                  Wo, bo, q_batch, k_batch):
    """Host fallback (degenerate batch layouts) + debugging aid."""
    Q = (q_feat @ Wq + bq).reshape(N, H, D).transpose(1, 0, 2)
    K = (k_feat @ Wk + bk).reshape(N, H, D).transpose(1, 0, 2)
    V = (v_feat @ Wv + bv).reshape(N, H, D).transpose(1, 0, 2)
    scores = np.einsum("hnd,hmd->hnm", Q, K) / SCALE + pos_enc
    mask = q_batch[:, None] != k_batch[None, :]
    scores = np.where(mask[None], np.float32(-1e9), scores)
    scores = scores - scores.max(-1, keepdims=True)
    e = np.exp(scores)
    probs = e / e.sum(-1, keepdims=True)
    h = np.einsum("hnm,hmd->hnd", probs, V)
    h = h.transpose(1, 0, 2).reshape(N, OD)
    return (h @ Wo + bo).astype(np.float32)


def kernel(q_feat, k_feat, v_feat, pos_enc, Wq, bq, Wk, bk, Wv, bv, Wo, bo,
           q_batch, k_batch):
    global LAST_RESULTS
    args = dict(q_feat=np.asarray(q_feat, np.float32),
                k_feat=np.asarray(k_feat, np.float32),
                v_feat=np.asarray(v_feat, np.float32),
                pos_enc=np.asarray(pos_enc, np.float32),
                Wq=np.asarray(Wq, np.float32), bq=np.asarray(bq, np.float32),
                Wk=np.asarray(Wk, np.float32), bk=np.asarray(bk, np.float32),
                Wv=np.asarray(Wv, np.float32), bv=np.asarray(bv, np.float32),
                Wo=np.asarray(Wo, np.float32), bo=np.asarray(bo, np.float32),
                q_batch=np.asarray(q_batch), k_batch=np.asarray(k_batch))

    qbound, kbound = _bounds(args["q_batch"], args["k_batch"])
    nq_all = np.diff(qbound)
    nk_all = np.diff(kbound)
    if np.any((nq_all > 0) & (nk_all == 0)):
        # some batch has queries but no keys: reference gives uniform attention
        # over ALL keys there; fall back (never happens with realistic inputs)
        return _kernel_numpy(**args)

    NQ = _r128(int(nq_all.max()))
    NK = _r128(int(nk_all.max()))

    has_bq = bool(np.any(args["bq"]))
    has_bk = bool(np.any(args["bk"]))
    has_bv = bool(np.any(args["bv"]))
    has_bo = bool(np.any(args["bo"]))

    nc = _build(NQ, NK, has_bq, has_bk, has_bv, has_bo)

    # ---- host-side sharding / layout / padding ----
    qfT = np.ascontiguousarray(args["q_feat"].T).astype(BF16_NP)
    kfT = np.ascontiguousarray(args["k_feat"].T).astype(BF16_NP)
    vfT = np.ascontiguousarray(args["v_feat"].T).astype(BF16_NP)
    wq8 = (args["Wq"] / SCALE).astype(BF16_NP)
    wkb = args["Wk"].astype(BF16_NP)
    wvb = args["Wv"].astype(BF16_NP)
    wob = np.ascontiguousarray(args["Wo"]).astype(BF16_NP)

    in_maps = []
    for c in range(NCORES):
        qs, qe = int(qbound[c]), int(qbound[c + 1])
        ks, ke = int(kbound[c]), int(kbound[c + 1])
        nq, nk = qe - qs, ke - ks

        qfc = np.zeros((QD, NQ), BF16_NP)
        qfc[:, :nq] = qfT[:, qs:qe]
        kfc = np.zeros((QD, NK), BF16_NP)
        kfc[:, :nk] = kfT[:, ks:ke]
        vfc = np.zeros((QD, NK), BF16_NP)
        vfc[:, :nk] = vfT[:, ks:ke]

        posc = np.full((H, NK, NQ), -1e9, BF16_NP)
        if nk > 0:
            posc[:, :nk, :] = 0.0   # real k rows: unmasked vs padded q cols
            posc[:, :nk, :nq] = args["pos_enc"][:, qs:qe, ks:ke] \
                .swapaxes(1, 2).astype(BF16_NP)

        m = {"qfT": qfc, "kfT": kfc, "vfT": vfc, "posc": posc,
             "wq": wq8, "wk": wkb, "wv": wvb, "wo": wob}
        if has_bq:
            m["bq"] = (args["bq"] / SCALE).astype(BF16_NP).reshape(1, OD)
        if has_bk:
            m["bk"] = args["bk"].astype(BF16_NP).reshape(1, OD)
        if has_bv:
            m["bv"] = args["bv"].astype(BF16_NP).reshape(1, OD)
        if has_bo:
            m["bo"] = np.ascontiguousarray(
                args["bo"].astype(np.float32).reshape(OD // 128, 128).T)
        in_maps.append(m)

    res = run_bass_kernel_spmd(nc, in_maps, core_ids=list(range(NCORES)),
                               trace=TRACE)
    LAST_RESULTS = res
    out = np.empty((N, OD), np.float32)
    for c in range(NCORES):
        qs, qe = int(qbound[c]), int(qbound[c + 1])
        if qe > qs:
            out[qs:qe, :] = res.results[c]["out"][:, :qe - qs].T
    return out


# revision 39
# speedup vs baseline: 3.2027x; 1.1225x over previous
"""Trainium2 Bass kernel for nn_MultiHeadAttention_3796751090171 (sparse_attention).

Batch-parallel SPMD across 8 NeuronCores: q_batch/k_batch are SORTED, so the
cross-batch mask makes attention block-diagonal over batches, and there are
exactly B=8 batches for 8 cores. Core c computes batch c's queries against
batch c's keys for ALL 8 heads -- completely independent work, so there are
NO collectives: the full output is a pure row-concatenation of the per-core
outputs.

Uniform SPMD template: every core runs the same program on [NKMAX x NQMAX]
padded tiles (NQMAX/NKMAX = max batch size rounded up to 128). The host pads
each core's feature slices with zeros and fabricates the pos_enc tile so that
  - padded k-rows carry pos = -1e9  -> exp = 0, no contribution to h or Z
  - padded q-cols carry pos = 0 on real k-rows (keeps Z finite; outputs for
    those columns are dropped by the host)

Per core c (batch slice qs:qe / ks:ke, all heads h):
  Q = qf[qs:qe] @ Wq/8, K = kf[ks:ke] @ Wk, V = vf[ks:ke] @ Wv  (+biases)
  per head: scoresT[k,q] = K_h^T-chunks @ Q_h + posT  (PSUM)
  expT = exp(scoresT); hT_unnorm/Z via [V|1] matmul (ones column -> row 64 = Z)
  hTn = hT * (1/Z broadcast); out^T[o,q] = sum_h Wo[64h:,:].T @ hTn_h + bo
Host: out[qs:qe, :] = outT[:, :nq].T

No max-subtraction in softmax: scores are O(10) so exp is safe in fp32; masked
entries give exp(-1e9+...) -> exactly 0, matching the reference's
exp(-1e9 - max) -> 0.
"""

import functools
import math

import numpy as np
import ml_dtypes

import concourse.bass as bass
import concourse.tile as tile
from concourse import bacc, mybir
from concourse.bass_utils import run_bass_kernel_spmd
from concourse.masks import make_identity

N = 3072
QD = 512
OD = 512
H = 8
D = 64
B = 8
NCORES = 8
SCALE = math.sqrt(D)

F32 = mybir.dt.float32
BF16 = mybir.dt.bfloat16
BF16_NP = ml_dtypes.bfloat16

TRACE = False
LAST_RESULTS = None


def _bounds(q_batch, k_batch):
    qb = np.asarray(q_batch).astype(np.int64)
    kb = np.asarray(k_batch).astype(np.int64)
    qbound = np.searchsorted(qb, np.arange(B + 1))
    kbound = np.searchsorted(kb, np.arange(B + 1))
    return qbound, kbound


def _chunks(lo, hi, step):
    return [(o, min(step, hi - o)) for o in range(lo, hi, step)]


def _r128(x):
    return max(128, ((x + 127) // 128) * 128)


@functools.lru_cache(maxsize=8)
def _build(NQ, NK, has_bq, has_bk, has_bv, has_bo):
    nc = bacc.Bacc("TRN2", target_bir_lowering=False, debug=False,
                   num_devices=NCORES)

    KT_T = QD // 128   # 4 contraction tiles for the projections
    NKC = NK // 128    # k chunks
    QCH = _chunks(0, NQ, 512)   # q chunks (free-dim <= 512)
    NTD = QD // 128    # output-d tiles for projections

    qfT_d = nc.dram_tensor("qfT", [QD, NQ], BF16, kind="ExternalInput")
    kfT_d = nc.dram_tensor("kfT", [QD, NK], BF16, kind="ExternalInput")
    vfT_d = nc.dram_tensor("vfT", [QD, NK], BF16, kind="ExternalInput")
    posc_d = nc.dram_tensor("posc", [H, NK, NQ], BF16, kind="ExternalInput")
    wq_d = nc.dram_tensor("wq", [QD, OD], BF16, kind="ExternalInput")
    wk_d = nc.dram_tensor("wk", [QD, OD], BF16, kind="ExternalInput")
    wv_d = nc.dram_tensor("wv", [QD, OD], BF16, kind="ExternalInput")
    wo_d = nc.dram_tensor("wo", [OD, OD], BF16, kind="ExternalInput")
    bq_d = nc.dram_tensor("bq", [1, OD], BF16, kind="ExternalInput") if has_bq else None
    bk_d = nc.dram_tensor("bk", [1, OD], BF16, kind="ExternalInput") if has_bk else None
    bv_d = nc.dram_tensor("bv", [1, OD], BF16, kind="ExternalInput") if has_bv else None
    bo_d = nc.dram_tensor("bo", [128, NTD], F32, kind="ExternalInput") if has_bo else None
    out_d = nc.dram_tensor("out", [OD, NQ], F32, kind="ExternalOutput")
    import os
    DEBUG = bool(os.environ.get("KDBG"))
    if DEBUG:
        dbg_h = nc.dram_tensor("dbg_h", [D, H, NQ], F32, kind="ExternalOutput")
        dbg_z = nc.dram_tensor("dbg_z", [1, H, NQ], F32, kind="ExternalOutput")
        dbg_e = nc.dram_tensor("dbg_e", [128, NQ], F32, kind="ExternalOutput")

    with tile.TileContext(nc) as tc:
        with (
            tc.tile_pool(name="consts", bufs=1) as consts,
            tc.tile_pool(name="pos", bufs=12) as posp,
            tc.tile_pool(name="expp", bufs=12) as expp,
            tc.tile_pool(name="outp", bufs=4) as outp,
            tc.tile_pool(name="ps_s", bufs=4, space="PSUM") as ps_s,
            tc.tile_pool(name="ps_h", bufs=2, space="PSUM") as ps_h,
            tc.tile_pool(name="ps_p", bufs=2, space="PSUM") as ps_p,
            tc.tile_pool(name="dram", bufs=1, space="DRAM") as dramp,
        ):
            # ---------------- constants / weights ----------------
            ones = consts.tile([1, max(NQ, NK)], BF16)
            nc.vector.memset(ones, 1.0)
            ones_f = consts.tile([1, D], F32)
            nc.vector.memset(ones_f, 1.0)
            ident128 = consts.tile([128, 128], BF16)
            make_identity(nc, ident128)

            wq_sb = consts.tile([128, KT_T, OD], BF16)
            wk_sb = consts.tile([128, KT_T, OD], BF16)
            wv_sb = consts.tile([128, KT_T, OD], BF16)
            for t_d, t_sb in ((wq_d, wq_sb), (wk_d, wk_sb), (wv_d, wv_sb)):
                nc.gpsimd.dma_start(
                    out=t_sb, in_=t_d.ap().rearrange("(t p) d -> p t d", p=128))
            # wo as [128, t, oc, 128] for contraction-128 output projection
            wo_sb = consts.tile([128, NTD, NTD, 128], BF16)
            nc.gpsimd.dma_start(
                out=wo_sb,
                in_=wo_d.ap().rearrange("(t p) (o c) -> p t o c", p=128, c=128))
            bias_sb = {}
            for nm, dd in (("bq", bq_d), ("bk", bk_d), ("bv", bv_d)):
                if dd is not None:
                    t = consts.tile([1, OD], BF16, tag=f"bias_{nm}", name=f"b_{nm}")
                    nc.gpsimd.dma_start(out=t, in_=dd[:, :])
                    bias_sb[nm] = t
            if bo_d is not None:
                bo_sb = consts.tile([128, NTD], F32)
                nc.gpsimd.dma_start(out=bo_sb, in_=bo_d[:, :])

            # feature tiles
            qf_sb = consts.tile([128, KT_T, NQ], BF16)
            kf_sb = consts.tile([128, KT_T, NK], BF16)
            vf_sb = consts.tile([128, KT_T, NK], BF16)
            for f_d, f_sb in ((qfT_d, qf_sb), (kfT_d, kf_sb), (vfT_d, vf_sb)):
                for t in range(KT_T):
                    nc.sync.dma_start(
                        out=f_sb[:, t, :],
                        in_=f_d.ap().rearrange("(t p) n -> t p n", p=128)[t])

            # projected tensors, split by head parity so every matmul operand
            # sits at partition base 0 (slot index = head // 2)
            QT_e = consts.tile([D, NTD, NQ], BF16, name="QT_e")
            QT_o = consts.tile([D, NTD, NQ], BF16, name="QT_o")
            KT_e = consts.tile([D, NTD, NK], BF16, name="KT_e")
            KT_o = consts.tile([D, NTD, NK], BF16, name="KT_o")
            VT_f = consts.tile([128, NTD, NK], BF16, name="VT_f")
            V_sb = consts.tile([128, NKC, H, D + 1], BF16, name="V_sb")
            hT_sb = consts.tile([D, H, NQ], BF16, name="hT_sb")
            hTn_sb = consts.tile([128, NTD, NQ], BF16, name="hTn_sb")
            zall_sb = consts.tile([1, H, NQ], F32, name="zall_sb")
            zrec_sb = consts.tile([1, H, NQ], F32, name="zrec_sb")

            # ---------------- projections ----------------
            def project(f_sb, w_sb, bias, dstE, dstO, xchunks):
                for td in range(NTD):
                    dsl = slice(128 * td, 128 * (td + 1))
                    for (xo, xw) in xchunks:
                        xsl = slice(xo, xo + xw)
                        psum = ps_p.tile([128, 512], F32, tag="psp")
                        for t in range(KT_T):
                            nc.tensor.matmul(psum[:, 0:xw],
                                             w_sb[:, t, dsl], f_sb[:, t, xsl],
                                             start=(t == 0),
                                             stop=(t == KT_T - 1 and bias is None))
                        if bias is not None:
                            nc.tensor.matmul(psum[:, 0:xw], bias[:, dsl],
                                             ones[:, xsl], start=False, stop=True)
                        nc.vector.tensor_copy(dstE[:, td, xsl], psum[0:D, 0:xw])
                        nc.scalar.copy(dstO[:, td, xsl], psum[D:128, 0:xw])

            kchunks = _chunks(0, NK, 512)
            project(qf_sb, wq_sb, bias_sb.get("bq"), QT_e, QT_o, QCH)
            project(kf_sb, wk_sb, bias_sb.get("bk"), KT_e, KT_o, kchunks)

            def project_full(f_sb, w_sb, bias, dst, xchunks):
                for td in range(NTD):
                    dsl = slice(128 * td, 128 * (td + 1))
                    for (xo, xw) in xchunks:
                        xsl = slice(xo, xo + xw)
                        psum = ps_p.tile([128, 512], F32, tag="psp")
                        for t in range(KT_T):
                            nc.tensor.matmul(psum[:, 0:xw],
                                             w_sb[:, t, dsl], f_sb[:, t, xsl],
                                             start=(t == 0),
                                             stop=(t == KT_T - 1 and bias is None))
                        if bias is not None:
                            nc.tensor.matmul(psum[:, 0:xw], bias[:, dsl],
                                             ones[:, xsl], start=False, stop=True)
                        nc.scalar.copy(dst[:, td, xsl], psum[:, 0:xw])

            project_full(vf_sb, wv_sb, bias_sb.get("bv"), VT_f, kchunks)

            # V into [k, d | ones] per (kchunk, d-tile) via full PE transposes;
            # each [128,128] transpose covers two heads' 64-dim halves
            nc.vector.memset(V_sb[:, :, :, D], 1.0)
            for kc in range(NKC):
                ksl = slice(128 * kc, 128 * (kc + 1))
                for td in range(NTD):
                    pst = ps_p.tile([128, 512], BF16, tag="psp")
                    nc.tensor.transpose(pst[:, 0:128], VT_f[:, td, ksl],
                                        ident128[:, :])
                    nc.scalar.copy(V_sb[:, kc, 2 * td, 0:D], pst[:, 0:D])
                    nc.vector.tensor_copy(V_sb[:, kc, 2 * td + 1, 0:D],
                                          pst[:, D:128])

            # ---------------- attention, software-pipelined ----------------
            units = [(h, qo, qw) for h in range(H) for (qo, qw) in QCH]
            expts = {}

            def stage1(i):
                h, qo, qw = units[i]
                qsl = slice(qo, qo + qw)
                QT = QT_e if h % 2 == 0 else QT_o
                KT = KT_e if h % 2 == 0 else KT_o
                lst = []
                for kc in range(NKC):
                    ksl = slice(128 * kc, 128 * (kc + 1))
                    ps = ps_s.tile([128, 512], F32, tag="pss")
                    nc.tensor.matmul(ps[:, 0:qw], KT[:, h // 2, ksl],
                                     QT[:, h // 2, qsl], start=True, stop=True)
                    pos = posp.tile([128, 512], BF16, tag="pos")
                    nc.gpsimd.dma_start(out=pos[:, 0:qw], in_=posc_d[h, ksl, qsl])
                    nc.vector.tensor_add(ps[:, 0:qw], ps[:, 0:qw], pos[:, 0:qw])
                    expt = expp.tile([128, 512], BF16, tag="expt")
                    nc.scalar.activation(expt[:, 0:qw], ps[:, 0:qw],
                                         mybir.ActivationFunctionType.Exp)
                    if DEBUG and i == 0 and kc == 0:
                        de = consts.tile([128, NQ], F32, name="de")
                        nc.vector.tensor_copy(de[:, 0:qw], expt[:, 0:qw])
                        nc.gpsimd.dma_start(out=dbg_e.ap(), in_=de[:, :])
                    lst.append(expt)
                expts[i] = lst

            def stage2(i):
                h, qo, qw = units[i]
                qsl = slice(qo, qo + qw)
                psum_h = ps_h.tile([D + 1, 512], F32, tag="psh")
                for kc in range(NKC):
                    nc.tensor.matmul(psum_h[:, 0:qw], V_sb[:, kc, h, :],
                                     expts[i][kc][:, 0:qw],
                                     start=(kc == 0), stop=(kc == NKC - 1))
                del expts[i]
                nc.scalar.copy(hT_sb[:, h, qsl], psum_h[0:D, 0:qw])
                # 1/Z for this unit (Z = row 64 of the accumulator); the
                # approx reciprocal requires an SBUF source, so copy first
                nc.scalar.copy(zall_sb[0:1, h, qsl], psum_h[D:D + 1, 0:qw])
                nc.vector.reciprocal_approx_fast(zrec_sb[0:1, h, qsl],
                                                 zall_sb[0:1, h, qsl])
                # broadcast 1/Z across the 64 d-partitions via a DRAM
                # bounce + stride-0 partition read, then normalize hT
                zr_d = dramp.tile([1, 512], F32, tag=f"zrd{i}", name=f"zrd{i}")
                nc.gpsimd.dma_start(out=zr_d[0:1, 0:qw],
                                    in_=zrec_sb[0:1, h, qsl])
                zr_ap = zr_d[:, :]
                zbc_src = bass.AP(tensor=zr_ap.tensor, offset=zr_ap.offset,
                                  ap=[[0, D], [1, qw]])
                zbc = posp.tile([D, 512], F32, tag="zbc", name="zbc")
                nc.gpsimd.dma_start(out=zbc[:, 0:qw], in_=zbc_src)
                po = D * (h % 2)
                nc.vector.tensor_mul(hTn_sb[po:po + D, h // 2, qsl],
                                     hT_sb[:, h, qsl], zbc[:, 0:qw])

            LOOK = 2
            for i in range(len(units)):
                stage1(i)
                if i >= LOOK:
                    stage2(i - LOOK)
            for i in range(max(0, len(units) - LOOK), len(units)):
                stage2(i)

            if DEBUG:
                dh = consts.tile([D, H, NQ], F32, name="dh")
                nc.vector.tensor_copy(dh[:, :, :], hT_sb[:, :, :])
                nc.gpsimd.dma_start(out=dbg_h.ap(), in_=dh[:, :, :])
                nc.gpsimd.dma_start(out=dbg_z.ap(), in_=zrec_sb[0:1, :, :])

            # ---------------- output projection ----------------
            for oc in range(NTD):
                for (qo, qw) in QCH:
                    qsl = slice(qo, qo + qw)
                    psum = ps_p.tile([128, 512], F32, tag="psp")
                    for t in range(NTD):
                        nc.tensor.matmul(psum[:, 0:qw], wo_sb[:, t, oc, :],
                                         hTn_sb[:, t, qsl],
                                         start=(t == 0), stop=(t == NTD - 1))
                    o_sb = outp.tile([128, 512], F32, tag="osb")
                    if bo_d is not None:
                        nc.scalar.activation(o_sb[:, 0:qw], psum[:, 0:qw],
                                             mybir.ActivationFunctionType.Identity,
                                             bias=bo_sb[:, oc:oc + 1])
                    else:
                        nc.vector.tensor_copy(o_sb[:, 0:qw], psum[:, 0:qw])
                    nc.gpsimd.dma_start(out=out_d[128 * oc:128 * (oc + 1), qsl],
                                        in_=o_sb[:, 0:qw])

    nc.compile()
    return nc


def _kernel_numpy(q_feat, k_feat, v_feat, pos_enc, Wq, bq, Wk, bk, Wv, bv,
                  Wo, bo, q_batch, k_batch):
    """Host fallback (degenerate batch layouts) + debugging aid."""
    Q = (q_feat @ Wq + bq).reshape(N, H, D).transpose(1, 0, 2)
    K = (k_feat @ Wk + bk).reshape(N, H, D).transpose(1, 0, 2)
    V = (v_feat @ Wv + bv).reshape(N, H, D).transpose(1, 0, 2)
    scores = np.einsum("hnd,hmd->hnm", Q, K) / SCALE + pos_enc
    mask = q_batch[:, None] != k_batch[None, :]
    scores = np.where(mask[None], np.float32(-1e9), scores)
    scores = scores - scores.max(-1, keepdims=True)
    e = np.exp(scores)
    probs = e / e.sum(-1, keepdims=True)
    h = np.einsum("hnm,hmd->hnd", probs, V)
    h = h.transpose(1, 0, 2).reshape(N, OD)
    return (h @ Wo + bo).astype(np.float32)


def kernel(q_feat, k_feat, v_feat, pos_enc, Wq, bq, Wk, bk, Wv, bv, Wo, bo,
           q_batch, k_batch):
    global LAST_RESULTS
    args = dict(q_feat=np.asarray(q_feat, np.float32),
                k_feat=np.asarray(k_feat, np.float32),
                v_feat=np.asarray(v_feat, np.float32),
                pos_enc=np.asarray(pos_enc, np.float32),
                Wq=np.asarray(Wq, np.float32), bq=np.asarray(bq, np.float32),
                Wk=np.asarray(Wk, np.float32), bk=np.asarray(bk, np.float32),
                Wv=np.asarray(Wv, np.float32), bv=np.asarray(bv, np.float32),
                Wo=np.asarray(Wo, np.float32), bo=np.asarray(bo, np.float32),
                q_batch=np.asarray(q_batch), k_batch=np.asarray(k_batch))

    qbound, kbound = _bounds(args["q_batch"], args["k_batch"])
    nq_all = np.diff(qbound)
    nk_all = np.diff(kbound)
    if np.any((nq_all > 0) & (nk_all == 0)):
        # some batch has queries but no keys: reference gives uniform attention
        # over ALL keys there; fall back (never happens with realistic inputs)
        return _kernel_numpy(**args)

    NQ = _r128(int(nq_all.max()))
    NK = _r128(int(nk_all.max()))

    has_bq = bool(np.any(args["bq"]))
    has_bk = bool(np.any(args["bk"]))
    has_bv = bool(np.any(args["bv"]))
    has_bo = bool(np.any(args["bo"]))

    nc = _build(NQ, NK, has_bq, has_bk, has_bv, has_bo)

    # ---- host-side sharding / layout / padding ----
    qfT = np.ascontiguousarray(args["q_feat"].T).astype(BF16_NP)
    kfT = np.ascontiguousarray(args["k_feat"].T).astype(BF16_NP)
    vfT = np.ascontiguousarray(args["v_feat"].T).astype(BF16_NP)
    wq8 = (args["Wq"] / SCALE).astype(BF16_NP)
    wkb = args["Wk"].astype(BF16_NP)
    wvb = args["Wv"].astype(BF16_NP)
    wob = np.ascontiguousarray(args["Wo"]).astype(BF16_NP)

    in_maps = []
    for c in range(NCORES):
        qs, qe = int(qbound[c]), int(qbound[c + 1])
        ks, ke = int(kbound[c]), int(kbound[c + 1])
        nq, nk = qe - qs, ke - ks

        qfc = np.zeros((QD, NQ), BF16_NP)
        qfc[:, :nq] = qfT[:, qs:qe]
        kfc = np.zeros((QD, NK), BF16_NP)
        kfc[:, :nk] = kfT[:, ks:ke]
        vfc = np.zeros((QD, NK), BF16_NP)
        vfc[:, :nk] = vfT[:, ks:ke]

        posc = np.full((H, NK, NQ), -1e9, BF16_NP)
        if nk > 0:
            posc[:, :nk, :] = 0.0   # real k rows: unmasked vs padded q cols
            posc[:, :nk, :nq] = args["pos_enc"][:, qs:qe, ks:ke] \
                .swapaxes(1, 2).astype(BF16_NP)

        m = {"qfT": qfc, "kfT": kfc, "vfT": vfc, "posc": posc,
             "wq": wq8, "wk": wkb, "wv": wvb, "wo": wob}
        if has_bq:
            m["bq"] = (args["bq"] / SCALE).astype(BF16_NP).reshape(1, OD)
        if has_bk:
            m["bk"] = args["bk"].astype(BF16_NP).reshape(1, OD)
        if has_bv:
            m["bv"] = args["bv"].astype(BF16_NP).reshape(1, OD)
        if has_bo:
            m["bo"] = np.ascontiguousarray(
                args["bo"].astype(np.float32).reshape(OD // 128, 128).T)
        in_maps.append(m)

    res = run_bass_kernel_spmd(nc, in_maps, core_ids=list(range(NCORES)),
                               trace=TRACE)
    LAST_RESULTS = res
    out = np.empty((N, OD), np.float32)
    for c in range(NCORES):
        qs, qe = int(qbound[c]), int(qbound[c + 1])
        if qe > qs:
            out[qs:qe, :] = res.results[c]["out"][:, :qe - qs].T
    return out
